# revision 1
# baseline (speedup 1.0000x reference)
"""Bilinear pooling kernel for 8 Trainium2 NeuronCores (Bass/Tile).

Math (matches the jax reference):
  x = concat([x1, x2, x3], channel) -> (B=64, M=147, L=3136)
  phi_b = x_b @ x_b.T                              (147, 147), symmetric
  phi = sign(phi) * sqrt(|phi| + EPS)              (signed sqrt)
  phi = phi / sqrt(sum(phi^2 + EPS) + 1.0)         (per-batch normalize)
  h = phi_vec @ fc0_w.T + fc0_b                    (64, 1024)
  y = h @ fc1_w.T + fc1_b                          (64, 64)
  logits = y @ fc2_w.T + fc2_b                     (64, 4)
  merged = softmax(concat([logits, x11, x21, x31]))
  x_merge = merged @ cls_w.T + cls_b               (64, 4)
  returns (logits, x_merge)

Distribution:
  phase 0: preload all fc0 weights + tail constants (overlaps phase 1)
  phase 1: batch-parallel bilinear+signed-sqrt+normalize (8 batches/core)
  phase 2: AllGather normalized phi (padded to 21632 cols)
  phase 3: PE-transpose phi to (i, b) layout; fc0 with output-column-sharded
           W^T (128 of 1024 outputs per core); fc1 partial contraction
  phase 4: AllReduce the (64, 64) y^T partials
  phase 5: replicated tail (fc2, softmax, cls); outputs read from core 0

MM_DT selects the matmul dtype for the two big GEMMs (bilinear + fc0);
everything else (signed sqrt, normalize, fc1/fc2/softmax/cls) stays fp32.
"""

import sys

sys.path.insert(0, "/opt/trn_rl_repo")

import numpy as np

import concourse.bass as bass
import concourse.tile as tile
from concourse import masks, mybir
from concourse.bass_utils import run_bass_kernel_spmd
import bass_rust
from bass_rust import ScopedClock

# ---------------------------------------------------------------------------
# Workaround: this toolchain's walrus accepts only ONE semaphore wait per
# instruction, but Tile can attach several.  Split excess waits onto
# same-engine nops placed immediately before the instruction (same engine
# => executed in order, so synchronization semantics are unchanged).
# ---------------------------------------------------------------------------
_MAX_WAITS = 1
_ws_counter = [0]


def _split_excess_waits(obb):
    for bb, insts in list(obb.items()):
        new_list = []
        for inst in insts:
            info = inst.sync_info
            if info is not None and len(info.on_wait) > _MAX_WAITS:
                waits = list(info.on_wait)
                excess = waits[:-_MAX_WAITS]
                keep = waits[-_MAX_WAITS:]
                for i in range(0, len(excess), _MAX_WAITS):
                    _ws_counter[0] += 1
                    nop = mybir.InstNoOp(
                        name=f"WS-{_ws_counter[0]}",
                        sync_info=bass_rust.SyncInfo(
                            on_wait=excess[i : i + _MAX_WAITS], on_update=[]
                        ),
                        bass_nofuse=True,
                        engine=inst.engine,
                    )
                    new_list.append(nop)
                inst.sync_info = bass_rust.SyncInfo(
                    on_wait=keep, on_update=list(info.on_update)
                )
            new_list.append(inst)
        obb[bb] = new_list


_RealTCW = tile.TileClockWait


class _TCWWrapper:
    def __init__(self, *args, **kwargs):
        self._inner = _RealTCW(*args, **kwargs)
        self._obb = (
            args[1] if len(args) > 1 else kwargs["ordered_instructions_by_block"]
        )

    def __getattr__(self, name):
        return getattr(self._inner, name)

    def assign_waits(self, bb_name):
        self._inner.assign_waits(bb_name)
        _split_excess_waits(self._obb)


tile.TileClockWait = _TCWWrapper


def _split_drain_and_barrier(self, tick_clock, wait_clock):
    nc = self.nc
    drain_inst = nc.sync.drain()
    wait_clock.add_sem_waits(
        drain_inst.ins, ScopedClock({None: tick_clock.global_clock})
    )
    info = drain_inst.ins.sync_info
    if info is not None and len(info.on_wait) > _MAX_WAITS:
        waits = list(info.on_wait)
        drain_inst.ins.sync_info = bass_rust.SyncInfo(
            on_wait=waits[:_MAX_WAITS], on_update=list(info.on_update)
        )
        rest = waits[_MAX_WAITS:]
        while rest:
            chunk, rest = rest[:_MAX_WAITS], rest[_MAX_WAITS:]
            nop_inst = nc.sync.nop(nofuse=True, hint="tail_drain_split")
            nop_inst.ins.sync_info = bass_rust.SyncInfo(on_wait=chunk, on_update=[])
    nc.all_engine_barrier()
    assert self.sems is not None
    popped = nc._tile_sem_poison_stack.pop()
    assert popped is self._sem_poison
    nc.clear_and_free_semaphores(list(self.sems.allocated().values()))
    nc.all_engine_barrier()


tile.TileContext._drain_and_barrier = _split_drain_and_barrier

# ---------------------------------------------------------------------------
# Problem constants (hardcoded per the spec)
# ---------------------------------------------------------------------------
N_CORES = 8
CORE_IDS = list(range(N_CORES))
B = 64
B_LOC = B // N_CORES  # 8 batches per core
C = 49
L = 3136  # 56*56
M = 147  # 3*49 channels
MM = M * M  # 21609
NI_CHUNKS = 169  # ceil(MM/128)
MM_PAD = NI_CHUNKS * 128  # 21632
O0 = 1024  # fc0 out features
O0_LOC = O0 // N_CORES  # 128 per core
HID = 64  # fc1 out features
CLS = 4
EPS = 1e-8
# normalizer constant: sum(phi_ss^2 + EPS) + 1.0 == sum|phi| + 2*MM*EPS + 1.0
NORM_C = float(2 * MM * EPS + 1.0)

LFULL = 24  # full 128-row l-chunks
LTAIL = 64  # tail chunk rows (3136 = 24*128 + 64)

F32 = mybir.dt.float32

# matmul dtype for the two big GEMMs: "float32", "bfloat16", or "float16"
MM_DT_NAME = "float16"
MM_DT = getattr(mybir.dt, MM_DT_NAME)
W_DMA = 8  # i-chunks per fc0 weight DMA


def _build_nc():
    nc = bass.Bass()

    # -- external I/O ------------------------------------------------------
    # x arrives host-side concatenated over channels and transposed to
    # (b, l, m) so device loads are contiguous along the innermost dim.
    xall_d = nc.dram_tensor("xall", [B_LOC, L, M], MM_DT, kind="ExternalInput")
    x11_d = nc.dram_tensor("x11", [B, CLS], F32, kind="ExternalInput")
    x21_d = nc.dram_tensor("x21", [B, CLS], F32, kind="ExternalInput")
    x31_d = nc.dram_tensor("x31", [B, CLS], F32, kind="ExternalInput")
    w0t_d = nc.dram_tensor("w0t", [MM_PAD, O0_LOC], MM_DT, kind="ExternalInput")
    fc0b_d = nc.dram_tensor("fc0b", [O0_LOC, 1], F32, kind="ExternalInput")
    w1t_d = nc.dram_tensor("w1t", [O0_LOC, HID], F32, kind="ExternalInput")
    fc1b_d = nc.dram_tensor("fc1b", [HID, 1], F32, kind="ExternalInput")
    w2t_d = nc.dram_tensor("w2t", [HID + 1, CLS], F32, kind="ExternalInput")
    wct_d = nc.dram_tensor("wct", [4 * CLS + 1, CLS], F32, kind="ExternalInput")
    logits_d = nc.dram_tensor("logits", [B, CLS], F32, kind="ExternalOutput")
    xmerge_d = nc.dram_tensor("x_merge", [B, CLS], F32, kind="ExternalOutput")

    n_wdma = (NI_CHUNKS + W_DMA - 1) // W_DMA  # 22 (last has 1 chunk)

    with tile.TileContext(nc) as tc:
        with tc.tile_pool(name="dram", bufs=1, space="DRAM") as dram, tc.tile_pool(
            name="const", bufs=1
        ) as const:
            # -- collective buffers (phi gathered in two b-halves so the
            # first AllGather overlaps the second half of phase 1) --------
            phi_cont_a = dram.tile([B_LOC // 2, MM_PAD], MM_DT)
            phi_cont_b = dram.tile([B_LOC // 2, MM_PAD], MM_DT)
            phi_all_a = dram.tile([B // 2, MM_PAD], MM_DT, addr_space="Shared")
            phi_all_b = dram.tile([B // 2, MM_PAD], MM_DT, addr_space="Shared")
            yt_part = dram.tile([HID, B], F32)
            yt_full = dram.tile([HID, B], F32, addr_space="Shared")

            # -- constants ----------------------------------------------
            identf = const.tile([128, 128], F32)
            masks.make_identity(nc, identf[:])
            if MM_DT != F32:
                ident = const.tile([128, 128], MM_DT)
                masks.make_identity(nc, ident[:])
            else:
                ident = identf
            ones_col = const.tile([128, 128], F32)
            nc.gpsimd.memset(ones_col[:], 1.0)
            tail_pat = const.tile([1, MM_PAD - MM], MM_DT)
            nc.gpsimd.memset(tail_pat[:], 0.0)
            eps_col = const.tile([128, 1], F32)
            nc.gpsimd.memset(eps_col[:], EPS)
            normc_col = const.tile([128, 1], F32)
            nc.gpsimd.memset(normc_col[:], NORM_C)

            # ===========================================================
            # phase 0: preload fc0 weights + small tail tensors (no deps,
            # so these DMAs overlap phase-1 compute)
            # ===========================================================
            w_sb = const.tile([128, NI_CHUNKS, O0_LOC], MM_DT)
            for wd in range(n_wdma):
                k0 = wd * W_DMA
                kn = min(W_DMA, NI_CHUNKS - k0)
                nc.scalar.dma_start(
                    w_sb[:, k0 : k0 + kn, :],
                    w0t_d[128 * k0 : 128 * (k0 + kn)].rearrange(
                        "(kc p) o -> p kc o", p=128
                    ),
                )
            fc0b_sb = const.tile([O0_LOC, 1], F32)
            nc.sync.dma_start(fc0b_sb[:], fc0b_d[:])
            w1_sb = const.tile([O0_LOC, HID], F32)
            nc.sync.dma_start(w1_sb[:], w1t_d[:])
            fc1b_sb = const.tile([HID, 1], F32)
            nc.sync.dma_start(fc1b_sb[:], fc1b_d[:])
            w2_sb = const.tile([HID + 1, CLS], F32)
            nc.sync.dma_start(w2_sb[:], w2t_d[:])
            wc_sb = const.tile([4 * CLS + 1, CLS], F32)
            nc.sync.dma_start(wc_sb[:], wct_d[:])
            xm1_sb = const.tile([B, CLS], F32)
            nc.sync.dma_start(xm1_sb[:], x11_d[:])
            xm2_sb = const.tile([B, CLS], F32)
            nc.sync.dma_start(xm2_sb[:], x21_d[:])
            xm3_sb = const.tile([B, CLS], F32)
            nc.sync.dma_start(xm3_sb[:], x31_d[:])
            # pre-staged tail tiles (written once, reused in phase 5)
            yt_aug = const.tile([HID + 1, B], F32)
            nc.vector.tensor_copy(yt_aug[HID : HID + 1, :], ones_col[0:1, 0:B])
            merged = const.tile([B, 4 * CLS], F32)
            nc.vector.tensor_copy(merged[:, CLS : 2 * CLS], xm1_sb[:])
            nc.vector.tensor_copy(merged[:, 2 * CLS : 3 * CLS], xm2_sb[:])
            nc.vector.tensor_copy(merged[:, 3 * CLS : 4 * CLS], xm3_sb[:])

            # ===========================================================
            # phase 1: bilinear + signed sqrt + normalize, per batch
            # ===========================================================
            with tc.tile_pool(name="xt", bufs=2) as xt_pool, tc.tile_pool(
                name="p1sb", bufs=2
            ) as sb, tc.tile_pool(
                name="p1ps", bufs=2, space="PSUM"
            ) as ps, nc.named_scope("p1_bilinear"):

                def p1_mains(b):
                    # xt[p, lc, m] = x[b, 128*lc + p, m]
                    xt = xt_pool.tile([128, LFULL, M], MM_DT, tag="xt")
                    xtt = xt_pool.tile([LTAIL, M], MM_DT, tag="xtt")
                    nc.sync.dma_start(
                        xt[:],
                        xall_d[b][0 : 128 * LFULL].rearrange(
                            "(lc p) m -> p lc m", p=128
                        ),
                    )
                    nc.sync.dma_start(xtt[:], xall_d[b][128 * LFULL : L])

                    # phi row-blocks: A = rows 0:128, A2 = rows 128:147.
                    # Two separate consecutive accumulation passes: mixing
                    # two PSUM accumulation groups stalls the PE on every
                    # matmul (drain + weight reload between groups).
                    pA = ps.tile([128, M], F32, tag="pA", bufs=3)
                    pB = ps.tile([M - 128, M], F32, tag="pB", bufs=3)
                    for lc in range(LFULL + 1):
                        lhs_a = xt[:, lc, 0:128] if lc < LFULL else xtt[:, 0:128]
                        rhs_a = xt[:, lc, :] if lc < LFULL else xtt[:, :]
                        nc.tensor.matmul(
                            pA[:], lhs_a, rhs_a, start=(lc == 0), stop=(lc == LFULL)
                        )
                    for lc in range(LFULL + 1):
                        lhs_b = xt[:, lc, 128:M] if lc < LFULL else xtt[:, 128:M]
                        rhs_a = xt[:, lc, :] if lc < LFULL else xtt[:, :]
                        nc.tensor.matmul(
                            pB[:], lhs_b, rhs_a, start=(lc == 0), stop=(lc == LFULL)
                        )
                    return pA, pB

                def p1_norm(b, pA, pB):
                    # signed sqrt pieces
                    sgnA = sb.tile([128, M], F32, tag="sgnA")
                    absA = sb.tile([128, M], F32, tag="absA")
                    sgnB = sb.tile([M - 128, M], F32, tag="sgnB")
                    absB = sb.tile([M - 128, M], F32, tag="absB")
                    nc.scalar.activation(
                        sgnA[:], pA[:], mybir.ActivationFunctionType.Sign
                    )
                    nc.scalar.activation(
                        absA[:], pA[:], mybir.ActivationFunctionType.Abs
                    )
                    nc.scalar.activation(
                        sgnB[:], pB[:], mybir.ActivationFunctionType.Sign
                    )
                    nc.scalar.activation(
                        absB[:], pB[:], mybir.ActivationFunctionType.Abs
                    )

                    # row sums of |phi| for the normalizer
                    rsA = sb.tile([128, 1], F32, tag="rsA")
                    rsB = sb.tile([M - 128, 1], F32, tag="rsB")
                    nc.vector.reduce_sum(rsA[:], absA[:], axis=mybir.AxisListType.X)
                    nc.vector.reduce_sum(rsB[:], absB[:], axis=mybir.AxisListType.X)

                    # ss = sign * sqrt(|phi| + EPS)
                    sqA = sb.tile([128, M], F32, tag="sqA")
                    sqB = sb.tile([M - 128, M], F32, tag="sqB")
                    nc.scalar.activation(
                        sqA[:],
                        absA[:],
                        mybir.ActivationFunctionType.Sqrt,
                        bias=eps_col[:],
                    )
                    nc.scalar.activation(
                        sqB[:],
                        absB[:],
                        mybir.ActivationFunctionType.Sqrt,
                        bias=eps_col[0 : M - 128],
                    )
                    ssA = sb.tile([128, M], F32, tag="ssA")
                    ssB = sb.tile([M - 128, M], F32, tag="ssB")
                    nc.vector.tensor_mul(ssA[:], sqA[:], sgnA[:])
                    nc.vector.tensor_mul(ssB[:], sqB[:], sgnB[:])

                    # cross-partition sum + broadcast in one accumulation
                    # group: bc[m] = sum_k ones[k, m] * rs[k]
                    bc = ps.tile([128, 1], F32, tag="bc")
                    nc.tensor.matmul(
                        bc[:], ones_col[:, :], rsA[:], start=True, stop=False
                    )
                    nc.tensor.matmul(
                        bc[:], ones_col[0 : M - 128, :], rsB[:], start=False, stop=True
                    )

                    # scale = 1 / sqrt(total + NORM_C)
                    inv = sb.tile([128, 1], F32, tag="inv")
                    nc.scalar.activation(
                        inv[:],
                        bc[:],
                        mybir.ActivationFunctionType.Sqrt,
                        bias=normc_col[:],
                    )
                    scl = sb.tile([128, 1], F32, tag="scl")
                    nc.vector.reciprocal(scl[:], inv[:])

                    # normalized phi, cast to MM_DT for the gather + fc0
                    nA = sb.tile([128, M], MM_DT, tag="nA")
                    nB = sb.tile([M - 128, M], MM_DT, tag="nB")
                    nc.vector.tensor_scalar_mul(nA[:], ssA[:], scl[:])
                    nc.vector.tensor_scalar_mul(nB[:], ssB[:], scl[0 : M - 128])

                    # write phi row (flattened, m-major) + zero pad tail
                    row = (phi_cont_a if b < B_LOC // 2 else phi_cont_b)[
                        b % (B_LOC // 2)
                    ]
                    nc.scalar.dma_start(
                        row[0 : 128 * M].rearrange("(m n) -> m n", n=M), nA[:]
                    )
                    nc.scalar.dma_start(
                        row[128 * M : MM].rearrange("(m n) -> m n", n=M), nB[:]
                    )
                    nc.scalar.dma_start(row[MM:MM_PAD], tail_pat[0, :])

                # 1-batch software pipeline: batch b's norm chain is issued
                # after batch b+1's matmuls, so the PE stream never stalls
                # waiting for the ACT/DVE chain
                prev = None
                for b in range(B_LOC):
                    cur = (b, *p1_mains(b))
                    if prev is not None:
                        p1_norm(*prev)
                    prev = cur
                p1_norm(*prev)

            # ===========================================================
            # phase 2: AllGather phi
            # ===========================================================
            with nc.named_scope("p2_allgather"):
                nc.gpsimd.collective_compute(
                    "AllGather",
                    mybir.AluOpType.bypass,
                    replica_groups=[CORE_IDS],
                    ins=[phi_cont_a.opt()],
                    outs=[phi_all_a.opt()],
                )
                nc.gpsimd.collective_compute(
                    "AllGather",
                    mybir.AluOpType.bypass,
                    replica_groups=[CORE_IDS],
                    ins=[phi_cont_b.opt()],
                    outs=[phi_all_b.opt()],
                )

            # ===========================================================
            # phase 3: transpose phi, fc0 (o-sharded), fc1 partial
            # ===========================================================
            with tc.tile_pool(name="p3sb", bufs=1) as sb3, tc.tile_pool(
                name="p3ps", bufs=2, space="PSUM"
            ) as ps3, tc.tile_pool(
                name="p3ph", bufs=1, space="PSUM"
            ) as psh, nc.named_scope("p3_fc0"):
                # phiT[p, k, j] = phi^T[128k + p, j] via xbar DMA-transpose,
                # in k-ranges per b-half so fc0 can start on early chunks
                phiT = sb3.tile([128, NI_CHUNKS, B], MM_DT)
                TK = 34
                for h, src_half in enumerate((phi_all_a, phi_all_b)):
                    for k0 in range(0, NI_CHUNKS, TK):
                        kk = min(TK, NI_CHUNKS - k0)
                        nc.sync.dma_start_transpose(
                            phiT[:, k0 : k0 + kk, 32 * h : 32 * (h + 1)],
                            src_half[:, 128 * k0 : 128 * (k0 + kk)],
                        )

                # fc0: h^T (128 o x 64 b), accumulate over 169 i-chunks
                ph = psh.tile([O0_LOC, B], F32)
                for k in range(NI_CHUNKS):
                    nc.tensor.matmul(
                        ph[:],
                        w_sb[:, k, :],
                        phiT[:, k, :],
                        start=(k == 0),
                        stop=(k == NI_CHUNKS - 1),
                    )

                # h = ph + fc0_b (exact fp32 bias add on the PSUM copy-out)
                h_sb = sb3.tile([O0_LOC, B], F32)
                nc.scalar.activation(
                    h_sb[:],
                    ph[:],
                    mybir.ActivationFunctionType.Identity,
                    bias=fc0b_sb[:],
                )

                # fc1 partial: y^T = w1t_shard.T @ h^T_shard
                py = ps3.tile([HID, B], F32, tag="py", bufs=1)
                nc.tensor.matmul(py[:], w1_sb[:], h_sb[:], start=True, stop=True)
                yt_sb = sb3.tile([HID, B], F32)
                nc.vector.tensor_copy(yt_sb[:], py[:])
                nc.sync.dma_start(yt_part[:], yt_sb[:])

            # ===========================================================
            # phase 4: AllReduce y^T partials
            # ===========================================================
            with nc.named_scope("p4_allreduce"):
                nc.gpsimd.collective_compute(
                    "AllReduce",
                    mybir.AluOpType.add,
                    replica_groups=[CORE_IDS],
                    ins=[yt_part.opt()],
                    outs=[yt_full.opt()],
                )

            # ===========================================================
            # phase 5: replicated tail
            # ===========================================================
            with tc.tile_pool(name="p5sb", bufs=1) as sb5, tc.tile_pool(
                name="p5ps", bufs=1, space="PSUM"
            ) as ps5, nc.named_scope("p5_tail"):
                # y^T + fc1_b (ones row pre-staged in phase 0)
                ytr = sb5.tile([HID, B], F32)
                nc.sync.dma_start(ytr[:], yt_full[:])
                nc.scalar.activation(
                    yt_aug[0:HID, :],
                    ytr[:],
                    mybir.ActivationFunctionType.Identity,
                    bias=fc1b_sb[:],
                )

                plog = ps5.tile([B, CLS], F32, tag="plog")
                nc.tensor.matmul(plog[:], yt_aug[:], w2_sb[:], start=True, stop=True)
                logit_sb = sb5.tile([B, CLS], F32)
                nc.scalar.copy(logit_sb[:], plog[:])
                # merged cols 4:16 pre-staged in phase 0 (x1i host-permuted
                # into gathered batch order); logits read from PSUM on DVE in
                # parallel with the ACT copy above
                nc.vector.tensor_copy(merged[:, 0:CLS], plog[:])
                # partition j holds global batch 8*(j%32//4) + 4*(j//32) + j%4
                # (b-halves gathered separately); undo it on the DMA write
                lview = logits_d.rearrange("(s e bl) c -> s e bl c", s=8, e=2)
                nc.sync.dma_start(lview[:, 0], logit_sb[0:32, :])
                nc.sync.dma_start(lview[:, 1], logit_sb[32:B, :])

                # softmax over the 16 features (free dim).  No max-subtract:
                # |merged| <= ~6 here, exp() is safely in range, and softmax
                # is shift-invariant so the result matches the reference.
                esb = sb5.tile([B, 4 * CLS], F32)
                ssum = sb5.tile([B, 1], F32)
                nc.scalar.activation(
                    esb[:],
                    merged[:],
                    mybir.ActivationFunctionType.Exp,
                    accum_out=ssum[:],
                )
                rinv = sb5.tile([B, 1], F32)
                nc.vector.reciprocal(rinv[:], ssum[:])

                # softmax result with a ones column appended (becomes the
                # bias row after the transpose)
                smx = sb5.tile([B, 4 * CLS + 1], F32)
                nc.vector.tensor_scalar_mul(smx[:, 0 : 4 * CLS], esb[:], rinv[:])
                nc.vector.tensor_copy(
                    smx[:, 4 * CLS : 4 * CLS + 1], ones_col[0:B, 0:1]
                )

                # x_merge = smx @ cls_w.T + cls_b  (via transposed smx + aug)
                pmt = ps5.tile([4 * CLS + 1, B], F32, tag="pmt")
                nc.tensor.transpose(pmt[:], smx[:], identf[0:B, 0:B])
                mt_aug = sb5.tile([4 * CLS + 1, B], F32)
                nc.scalar.copy(mt_aug[:], pmt[:])

                pxm = ps5.tile([B, CLS], F32, tag="pxm")
                nc.tensor.matmul(pxm[:], mt_aug[:], wc_sb[:], start=True, stop=True)
                xm_sb = sb5.tile([B, CLS], F32)
                nc.scalar.copy(xm_sb[:], pxm[:])
                xview = xmerge_d.rearrange("(s e bl) c -> s e bl c", s=8, e=2)
                nc.sync.dma_start(xview[:, 0], xm_sb[0:32, :])
                nc.sync.dma_start(xview[:, 1], xm_sb[32:B, :])

    return nc


_NC_CACHE = None


def _get_nc():
    global _NC_CACHE
    if _NC_CACHE is None:
        _NC_CACHE = _build_nc()
    return _NC_CACHE


def _make_in_maps(inputs):
    np_mm = np.dtype(mybir.dt.np(MM_DT))

    x1 = np.ascontiguousarray(inputs["x1"], dtype=np.float32).reshape(B, C, L)
    x2 = np.ascontiguousarray(inputs["x2"], dtype=np.float32).reshape(B, C, L)
    x3 = np.ascontiguousarray(inputs["x3"], dtype=np.float32).reshape(B, C, L)
    # (B, L, M) concatenated + transposed + cast
    xall = np.concatenate([x1, x2, x3], axis=1).transpose(0, 2, 1).astype(np_mm)
    # gathered batch order: columns j map to global batch
    # 8*(j%32//4) + 4*(j//32) + j%4
    perm = np.array(
        [8 * (s) + 4 * e + bl for e in range(2) for s in range(8) for bl in range(4)],
        dtype=np.int64,
    )
    x11 = np.ascontiguousarray(inputs["x11"], dtype=np.float32)[perm]
    x21 = np.ascontiguousarray(inputs["x21"], dtype=np.float32)[perm]
    x31 = np.ascontiguousarray(inputs["x31"], dtype=np.float32)[perm]
    fc0_w = np.asarray(inputs["fc0_w"], dtype=np.float32)
    fc0_b = np.asarray(inputs["fc0_b"], dtype=np.float32)
    fc1_w = np.asarray(inputs["fc1_w"], dtype=np.float32)
    fc1_b = np.asarray(inputs["fc1_b"], dtype=np.float32)
    fc2_w = np.asarray(inputs["fc2_w"], dtype=np.float32)
    fc2_b = np.asarray(inputs["fc2_b"], dtype=np.float32)
    cls_w = np.asarray(inputs["cls_w"], dtype=np.float32)
    cls_b = np.asarray(inputs["cls_b"], dtype=np.float32)

    w0t = np.zeros((MM_PAD, O0), dtype=np_mm)
    w0t[:MM] = fc0_w.T.astype(np_mm)
    w1t = np.ascontiguousarray(fc1_w.T)  # (1024, 64)
    fc1b = np.ascontiguousarray(fc1_b.reshape(HID, 1))
    w2t = np.ascontiguousarray(
        np.concatenate([fc2_w.T, fc2_b.reshape(1, CLS)], axis=0)
    )
    wct = np.ascontiguousarray(
        np.concatenate([cls_w.T, cls_b.reshape(1, CLS)], axis=0)
    )

    in_maps = []
    for c in range(N_CORES):
        sl = slice(B_LOC * c, B_LOC * (c + 1))
        ol = slice(O0_LOC * c, O0_LOC * (c + 1))
        in_maps.append(
            {
                "xall": np.ascontiguousarray(xall[sl]),
                "x11": x11,
                "x21": x21,
                "x31": x31,
                "w0t": np.ascontiguousarray(w0t[:, ol]),
                "fc0b": np.ascontiguousarray(fc0_b[ol].reshape(O0_LOC, 1)),
                "w1t": np.ascontiguousarray(w1t[ol]),
                "fc1b": fc1b,
                "w2t": w2t,
                "wct": wct,
            }
        )
    return in_maps


def run(inputs, trace=False, **kwargs):
    nc = _get_nc()
    in_maps = _make_in_maps(inputs)
    res = run_bass_kernel_spmd(nc, in_maps, CORE_IDS, trace=trace, **kwargs)
    out = res.results[0]
    logits = np.asarray(out["logits"], dtype=np.float32)
    x_merge = np.asarray(out["x_merge"], dtype=np.float32)
    return (logits, x_merge), res


def kernel(**inputs):
    (logits, x_merge), _ = run(inputs, trace=False)
    return logits, x_merge



# revision 5
# speedup vs baseline: 1.0405x; 1.0405x over previous
"""Bilinear pooling kernel for 8 Trainium2 NeuronCores (Bass/Tile).

Math (matches the jax reference):
  x = concat([x1, x2, x3], channel) -> (B=64, M=147, L=3136)
  phi_b = x_b @ x_b.T                              (147, 147), symmetric
  phi = sign(phi) * sqrt(|phi| + EPS)              (signed sqrt)
  phi = phi / sqrt(sum(phi^2 + EPS) + 1.0)         (per-batch normalize)
  h = phi_vec @ fc0_w.T + fc0_b                    (64, 1024)
  y = h @ fc1_w.T + fc1_b                          (64, 64)
  logits = y @ fc2_w.T + fc2_b                     (64, 4)
  merged = softmax(concat([logits, x11, x21, x31]))
  x_merge = merged @ cls_w.T + cls_b               (64, 4)
  returns (logits, x_merge)

Key tricks vs the naive layout:
  * all big DMAs are partition-contiguous (host pre-transposes), so each
    transfer is 128 descriptors of >=512B instead of thousands of ~280B ones
  * phi is symmetric: only the top 128x147 block (A) and the 19x19 corner
    are computed/gathered; fc0 weights are folded host-side
    (wA[m,n] = W0[m*147+n] + W0[n*147+m] for n>=128) so no transpose of phi
    is ever needed -- fc0 contracts A columns directly
  * AllGather of phi runs in 4 batch-pair chunks pipelined under phase 1;
    a dummy warm-up collective absorbs the CC cold-start latency

Distribution:
  phase 0: preload fc0 folded weights + tail constants (overlaps phase 1)
  phase 1: batch-parallel bilinear+signed-sqrt+normalize (8 batches/core),
           writing nA/corner to per-pair gather blobs
  phase 2: 4x pipelined AllGather (one per local batch-pair)
  phase 3: fc0 with output-column-sharded folded weights (128 of 1024 per
           core), 147+19 accumulating chunk matmuls; fc1 partial
  phase 4: AllReduce the (64, 64) y^T partials
  phase 5: replicated tail (fc2, softmax, cls); outputs read from core 0
"""

import sys

sys.path.insert(0, "/opt/trn_rl_repo")

import numpy as np

import concourse.bass as bass
import concourse.tile as tile
from concourse import masks, mybir
from concourse.bass_utils import run_bass_kernel_spmd
import bass_rust
from bass_rust import ScopedClock

# ---------------------------------------------------------------------------
# Workaround: this toolchain's walrus accepts only ONE semaphore wait per
# instruction, but Tile can attach several.  Split excess waits onto
# same-engine nops placed immediately before the instruction (same engine
# => executed in order, so synchronization semantics are unchanged).
# ---------------------------------------------------------------------------
_MAX_WAITS = 1
_ws_counter = [0]


def _split_excess_waits(obb):
    for bb, insts in list(obb.items()):
        new_list = []
        for inst in insts:
            info = inst.sync_info
            if info is not None and len(info.on_wait) > _MAX_WAITS:
                waits = list(info.on_wait)
                excess = waits[:-_MAX_WAITS]
                keep = waits[-_MAX_WAITS:]
                for i in range(0, len(excess), _MAX_WAITS):
                    _ws_counter[0] += 1
                    nop = mybir.InstNoOp(
                        name=f"WS-{_ws_counter[0]}",
                        sync_info=bass_rust.SyncInfo(
                            on_wait=excess[i : i + _MAX_WAITS], on_update=[]
                        ),
                        bass_nofuse=True,
                        engine=inst.engine,
                    )
                    new_list.append(nop)
                inst.sync_info = bass_rust.SyncInfo(
                    on_wait=keep, on_update=list(info.on_update)
                )
            new_list.append(inst)
        obb[bb] = new_list


_RealTCW = tile.TileClockWait


class _TCWWrapper:
    def __init__(self, *args, **kwargs):
        self._inner = _RealTCW(*args, **kwargs)
        self._obb = (
            args[1] if len(args) > 1 else kwargs["ordered_instructions_by_block"]
        )

    def __getattr__(self, name):
        return getattr(self._inner, name)

    def assign_waits(self, bb_name):
        self._inner.assign_waits(bb_name)
        _split_excess_waits(self._obb)


tile.TileClockWait = _TCWWrapper


def _split_drain_and_barrier(self, tick_clock, wait_clock):
    nc = self.nc
    drain_inst = nc.sync.drain()
    wait_clock.add_sem_waits(
        drain_inst.ins, ScopedClock({None: tick_clock.global_clock})
    )
    info = drain_inst.ins.sync_info
    if info is not None and len(info.on_wait) > _MAX_WAITS:
        waits = list(info.on_wait)
        drain_inst.ins.sync_info = bass_rust.SyncInfo(
            on_wait=waits[:_MAX_WAITS], on_update=list(info.on_update)
        )
        rest = waits[_MAX_WAITS:]
        while rest:
            chunk, rest = rest[:_MAX_WAITS], rest[_MAX_WAITS:]
            nop_inst = nc.sync.nop(nofuse=True, hint="tail_drain_split")
            nop_inst.ins.sync_info = bass_rust.SyncInfo(on_wait=chunk, on_update=[])
    nc.all_engine_barrier()
    assert self.sems is not None
    popped = nc._tile_sem_poison_stack.pop()
    assert popped is self._sem_poison
    nc.clear_and_free_semaphores(list(self.sems.allocated().values()))
    nc.all_engine_barrier()


tile.TileContext._drain_and_barrier = _split_drain_and_barrier

# ---------------------------------------------------------------------------
# Problem constants (hardcoded per the spec)
# ---------------------------------------------------------------------------
N_CORES = 8
CORE_IDS = list(range(N_CORES))
B = 64
B_LOC = B // N_CORES  # 8 batches per core
C = 49
L = 3136  # 56*56
LC = 25  # l-chunks of 128 (3200 rows, last 64 zero-padded)
M = 147  # 3*49 channels
MA = 128  # top block rows
MB = M - MA  # 19 corner rows
MM = M * M  # 21609
O0 = 1024  # fc0 out features
O0_LOC = O0 // N_CORES  # 128 per core
HID = 64  # fc1 out features
CLS = 4
EPS = 1e-8
# normalizer constant: sum(phi_ss^2 + EPS) + 1.0 == sum|phi| + 2*MM*EPS + 1.0
NORM_C = float(2 * MM * EPS + 1.0)

NG = 4  # gather chunks
PG = B_LOC // NG  # 2 local batches per gather
BLOB_A = MA * PG * M  # 37632 elems: [p=128][t=2][m=147]
BLOB_B = MB * PG * MB  # 722 elems:   [p=19][t=2][n=19]
BLOB = BLOB_A + BLOB_B  # 38354

F32 = mybir.dt.float32
MM_DT = mybir.dt.float16


def _build_nc():
    nc = bass.Bass()

    # -- external I/O ------------------------------------------------------
    # x arrives host-side concatenated, zero-padded to 3200 l-rows, and
    # pre-transposed to [b][p=128][lc=25][m=147] so each per-batch DMA is
    # 128 descriptors x 7350 B.
    xall_d = nc.dram_tensor("xall", [B_LOC, 128, LC, M], MM_DT, kind="ExternalInput")
    x11_d = nc.dram_tensor("x11", [B, CLS], F32, kind="ExternalInput")
    x21_d = nc.dram_tensor("x21", [B, CLS], F32, kind="ExternalInput")
    x31_d = nc.dram_tensor("x31", [B, CLS], F32, kind="ExternalInput")
    # folded fc0 weights: wA[m, n, o] = W0[o, m*147+n] (+ W0[o, n*147+m] for
    # n >= 128); w4[a, b4, o] = W0[o, (128+a)*147 + 128+b4]
    wA_d = nc.dram_tensor("wA", [MA, M, O0_LOC], MM_DT, kind="ExternalInput")
    w4_d = nc.dram_tensor("w4", [MB, MB, O0_LOC], MM_DT, kind="ExternalInput")
    fc0b_d = nc.dram_tensor("fc0b", [O0_LOC, 1], F32, kind="ExternalInput")
    w1t_d = nc.dram_tensor("w1t", [O0_LOC, HID], F32, kind="ExternalInput")
    fc1b_d = nc.dram_tensor("fc1b", [HID, 1], F32, kind="ExternalInput")
    w2t_d = nc.dram_tensor("w2t", [HID + 1, CLS], F32, kind="ExternalInput")
    wct_d = nc.dram_tensor("wct", [4 * CLS + 1, CLS], F32, kind="ExternalInput")
    logits_d = nc.dram_tensor("logits", [B, CLS], F32, kind="ExternalOutput")
    xmerge_d = nc.dram_tensor("x_merge", [B, CLS], F32, kind="ExternalOutput")

    with tile.TileContext(nc) as tc:
        with tc.tile_pool(name="dram", bufs=1, space="DRAM") as dram, tc.tile_pool(
            name="const", bufs=1
        ) as const:
            # -- collective buffers ------------------------------------
            blobs = [
                dram.tile([BLOB], MM_DT, name=f"blob{g}") for g in range(NG)
            ]
            gaths = [
                dram.tile([N_CORES, BLOB], MM_DT, addr_space="Shared", name=f"gath{g}")
                for g in range(NG)
            ]
            warm_in = dram.tile([64], MM_DT)
            warm_out = dram.tile([N_CORES * 64], MM_DT, addr_space="Shared")
            yt_part = dram.tile([HID, B], F32)
            yt_full = dram.tile([HID, B], F32, addr_space="Shared")

            # -- constants ----------------------------------------------
            identf = const.tile([128, 128], F32)
            masks.make_identity(nc, identf[:])
            ones_col = const.tile([128, 128], F32)
            nc.gpsimd.memset(ones_col[:], 1.0)
            eps_col = const.tile([128, 1], F32)
            nc.gpsimd.memset(eps_col[:], EPS)
            normc_col = const.tile([128, 1], F32)
            nc.gpsimd.memset(normc_col[:], NORM_C)
            warm_sb = const.tile([1, 64], MM_DT)
            nc.gpsimd.memset(warm_sb[:], 0.0)
            nc.gpsimd.dma_start(warm_in[:].rearrange("(o k) -> o k", o=1), warm_sb[:])

            # warm up the CC stream so the first real gather starts fast
            nc.gpsimd.collective_compute(
                "AllGather",
                mybir.AluOpType.bypass,
                replica_groups=[CORE_IDS],
                ins=[warm_in.opt()],
                outs=[warm_out.opt()],
            )

            # ===========================================================
            # phase 0: preload weights + small tail tensors (no deps, so
            # these DMAs overlap phase-1 compute)
            # ===========================================================
            w_sb = const.tile([MA, M, O0_LOC], MM_DT)
            nc.scalar.dma_start(w_sb[:], wA_d[:])
            w4_sb = const.tile([MB, MB, O0_LOC], MM_DT)
            nc.scalar.dma_start(w4_sb[:], w4_d[:])
            fc0b_sb = const.tile([O0_LOC, 1], F32)
            nc.scalar.dma_start(fc0b_sb[:], fc0b_d[:])
            w1_sb = const.tile([O0_LOC, HID], F32)
            nc.scalar.dma_start(w1_sb[:], w1t_d[:])
            fc1b_sb = const.tile([HID, 1], F32)
            nc.scalar.dma_start(fc1b_sb[:], fc1b_d[:])
            w2_sb = const.tile([HID + 1, CLS], F32)
            nc.scalar.dma_start(w2_sb[:], w2t_d[:])
            wc_sb = const.tile([4 * CLS + 1, CLS], F32)
            nc.scalar.dma_start(wc_sb[:], wct_d[:])
            xm1_sb = const.tile([B, CLS], F32)
            nc.scalar.dma_start(xm1_sb[:], x11_d[:])
            xm2_sb = const.tile([B, CLS], F32)
            nc.scalar.dma_start(xm2_sb[:], x21_d[:])
            xm3_sb = const.tile([B, CLS], F32)
            nc.scalar.dma_start(xm3_sb[:], x31_d[:])
            # pre-staged tail tiles (written once, reused in phase 5)
            yt_aug = const.tile([HID + 1, B], F32)
            nc.vector.tensor_copy(yt_aug[HID : HID + 1, :], ones_col[0:1, 0:B])
            merged = const.tile([B, 4 * CLS], F32)
            nc.vector.tensor_copy(merged[:, CLS : 2 * CLS], xm1_sb[:])
            nc.vector.tensor_copy(merged[:, 2 * CLS : 3 * CLS], xm2_sb[:])
            nc.vector.tensor_copy(merged[:, 3 * CLS : 4 * CLS], xm3_sb[:])

            # gathered phi blocks, SBUF-resident for fc0
            pallA = const.tile([MA, B, M], MM_DT)  # [p=m, cb, n]
            pallB = const.tile([MB, B, MB], MM_DT)  # [p=a, cb, b4]

            # ===========================================================
            # phase 1: bilinear + signed sqrt + normalize, per batch
            # ===========================================================
            with tc.tile_pool(name="xt", bufs=3) as xt_pool, tc.tile_pool(
                name="p1sb", bufs=2
            ) as sb, tc.tile_pool(
                name="p1ps", bufs=2, space="PSUM"
            ) as ps, nc.named_scope("p1_bilinear"):

                def p1_mains(b):
                    xt = xt_pool.tile([128, LC, M], MM_DT, tag="xt")
                    nc.sync.dma_start(xt[:], xall_d[b])
                    # A block: phi[0:128, 0:147]; corner: phi[128:147, 128:147]
                    pA = ps.tile([MA, M], F32, tag="pA", bufs=3)
                    pB2 = ps.tile([MB, MB], F32, tag="pB2", bufs=3)
                    for lc in range(LC):
                        nc.tensor.matmul(
                            pA[:],
                            xt[:, lc, 0:MA],
                            xt[:, lc, :],
                            start=(lc == 0),
                            stop=(lc == LC - 1),
                        )
                    for lc in range(LC):
                        nc.tensor.matmul(
                            pB2[:],
                            xt[:, lc, MA:M],
                            xt[:, lc, MA:M],
                            start=(lc == 0),
                            stop=(lc == LC - 1),
                        )
                    return pA, pB2

                def p1_norm(b, pA, pB2):
                    g, t = divmod(b, PG)
                    # signed sqrt pieces + |phi| row sums (accumulated on ACT)
                    sgnA = sb.tile([MA, M], F32, tag="sgnA")
                    absA = sb.tile([MA, M], F32, tag="absA")
                    rsA = sb.tile([MA, 1], F32, tag="rsA")
                    nc.scalar.activation(
                        sgnA[:], pA[:], mybir.ActivationFunctionType.Sign
                    )
                    nc.scalar.activation(
                        absA[:],
                        pA[:],
                        mybir.ActivationFunctionType.Abs,
                        accum_out=rsA[:],
                    )
                    sgnB2 = sb.tile([MB, MB], F32, tag="sgnB2")
                    absB2 = sb.tile([MB, MB], F32, tag="absB2")
                    rsB2 = sb.tile([MB, 1], F32, tag="rsB2")
                    nc.scalar.activation(
                        sgnB2[:], pB2[:], mybir.ActivationFunctionType.Sign
                    )
                    nc.scalar.activation(
                        absB2[:],
                        pB2[:],
                        mybir.ActivationFunctionType.Abs,
                        accum_out=rsB2[:],
                    )
                    # S2 columns counted twice (symmetric image lives in S3)
                    rsA2 = sb.tile([MA, 1], F32, tag="rsA2")
                    nc.vector.reduce_sum(
                        rsA2[:], absA[:, MA:M], axis=mybir.AxisListType.X
                    )
                    rsT = sb.tile([MA, 1], F32, tag="rsT")
                    nc.vector.tensor_add(rsT[:], rsA[:], rsA2[:])

                    # cross-partition sum + broadcast in one accumulation
                    # group: bc[m] = sum_k ones[k, m] * rs[k]
                    bc = ps.tile([128, 1], F32, tag="bc")
                    nc.tensor.matmul(
                        bc[:], ones_col[:, :], rsT[:], start=True, stop=False
                    )
                    nc.tensor.matmul(
                        bc[:], ones_col[0:MB, :], rsB2[:], start=False, stop=True
                    )

                    # ss = sign * sqrt(|phi| + EPS)
                    sqA = sb.tile([MA, M], F32, tag="sqA")
                    nc.scalar.activation(
                        sqA[:],
                        absA[:],
                        mybir.ActivationFunctionType.Sqrt,
                        bias=eps_col[:],
                    )
                    ssA = sb.tile([MA, M], F32, tag="ssA")
                    nc.vector.tensor_mul(ssA[:], sqA[:], sgnA[:])
                    sqB2 = sb.tile([MB, MB], F32, tag="sqB2")
                    nc.scalar.activation(
                        sqB2[:],
                        absB2[:],
                        mybir.ActivationFunctionType.Sqrt,
                        bias=eps_col[0:MB],
                    )
                    ssB2 = sb.tile([MB, MB], F32, tag="ssB2")
                    nc.vector.tensor_mul(ssB2[:], sqB2[:], sgnB2[:])

                    # scale = 1 / sqrt(total + NORM_C)
                    inv = sb.tile([128, 1], F32, tag="inv")
                    nc.scalar.activation(
                        inv[:],
                        bc[:],
                        mybir.ActivationFunctionType.Sqrt,
                        bias=normc_col[:],
                    )
                    scl = sb.tile([128, 1], F32, tag="scl")
                    nc.vector.reciprocal(scl[:], inv[:])

                    # normalized phi, cast to MM_DT for the gather + fc0
                    nA = sb.tile([MA, M], MM_DT, tag="nA")
                    nB2 = sb.tile([MB, MB], MM_DT, tag="nB2")
                    nc.vector.tensor_scalar_mul(nA[:], ssA[:], scl[0:MA])
                    nc.vector.tensor_scalar_mul(nB2[:], ssB2[:], scl[0:MB])

                    # write into this pair's gather blob (partition-major)
                    aview = blobs[g][0:BLOB_A].rearrange(
                        "(p t m) -> p t m", p=MA, t=PG
                    )
                    nc.scalar.dma_start(aview[:, t, :], nA[:])
                    bview = blobs[g][BLOB_A:BLOB].rearrange(
                        "(p t n) -> p t n", p=MB, t=PG
                    )
                    nc.scalar.dma_start(bview[:, t, :], nB2[:])

                # 1-batch software pipeline: batch b's norm chain is issued
                # after batch b+1's matmuls so the PE stream never stalls;
                # gather g fires as soon as its pair's blobs are written
                prev = None
                for b in range(B_LOC):
                    cur = (b, *p1_mains(b))
                    if prev is not None:
                        p1_norm(*prev)
                        pb = prev[0]
                        if pb % PG == PG - 1:
                            g = pb // PG
                            nc.gpsimd.collective_compute(
                                "AllGather",
                                mybir.AluOpType.bypass,
                                replica_groups=[CORE_IDS],
                                ins=[blobs[g].opt()],
                                outs=[gaths[g].opt()],
                            )
                    prev = cur
                p1_norm(*prev)
                nc.gpsimd.collective_compute(
                    "AllGather",
                    mybir.AluOpType.bypass,
                    replica_groups=[CORE_IDS],
                    ins=[blobs[NG - 1].opt()],
                    outs=[gaths[NG - 1].opt()],
                )

            # ===========================================================
            # phase 2: land gathered phi into SBUF (one DMA per gather)
            # ===========================================================
            with nc.named_scope("p2_land"):
                # cb = c*8 + g*2 + t  ->  pall view [p, c, g, t, n]
                pav = pallA[:].rearrange("p (c g t) n -> p c g t n", c=N_CORES, g=NG)
                pbv = pallB[:].rearrange("p (c g t) n -> p c g t n", c=N_CORES, g=NG)
                for g in range(NG):
                    nc.gpsimd.dma_start(
                        pav[:, :, g, :, :],
                        gaths[g][:, 0:BLOB_A].rearrange(
                            "c (p t m) -> p c t m", p=MA, t=PG
                        ),
                    )
                    nc.gpsimd.dma_start(
                        pbv[:, :, g, :, :],
                        gaths[g][:, BLOB_A:BLOB].rearrange(
                            "c (p t n) -> p c t n", p=MB, t=PG
                        ),
                    )

            # ===========================================================
            # phase 3: fc0 (o-sharded, folded weights) + fc1 partial
            # ===========================================================
            with tc.tile_pool(name="p3sb", bufs=1) as sb3, tc.tile_pool(
                name="p3ps", bufs=2, space="PSUM"
            ) as ps3, tc.tile_pool(
                name="p3ph", bufs=1, space="PSUM"
            ) as psh, nc.named_scope("p3_fc0"):
                ph = psh.tile([O0_LOC, B], F32)
                for n in range(M):
                    nc.tensor.matmul(
                        ph[:],
                        w_sb[:, n, :],
                        pallA[:, :, n],
                        start=(n == 0),
                        stop=False,
                    )
                for b4 in range(MB):
                    nc.tensor.matmul(
                        ph[:],
                        w4_sb[:, b4, :],
                        pallB[:, :, b4],
                        start=False,
                        stop=(b4 == MB - 1),
                    )

                # h = ph + fc0_b (exact fp32 bias add on the PSUM copy-out)
                h_sb = sb3.tile([O0_LOC, B], F32)
                nc.scalar.activation(
                    h_sb[:],
                    ph[:],
                    mybir.ActivationFunctionType.Identity,
                    bias=fc0b_sb[:],
                )

                # fc1 partial: y^T = w1t_shard.T @ h^T_shard
                py = ps3.tile([HID, B], F32, tag="py", bufs=1)
                nc.tensor.matmul(py[:], w1_sb[:], h_sb[:], start=True, stop=True)
                yt_sb = sb3.tile([HID, B], F32)
                nc.vector.tensor_copy(yt_sb[:], py[:])
                nc.sync.dma_start(yt_part[:], yt_sb[:])

            # ===========================================================
            # phase 4: AllReduce y^T partials
            # ===========================================================
            with nc.named_scope("p4_allreduce"):
                nc.gpsimd.collective_compute(
                    "AllReduce",
                    mybir.AluOpType.add,
                    replica_groups=[CORE_IDS],
                    ins=[yt_part.opt()],
                    outs=[yt_full.opt()],
                )

            # ===========================================================
            # phase 5: replicated tail
            # ===========================================================
            with tc.tile_pool(name="p5sb", bufs=1) as sb5, tc.tile_pool(
                name="p5ps", bufs=1, space="PSUM"
            ) as ps5, nc.named_scope("p5_tail"):
                # y^T + fc1_b (ones row pre-staged in phase 0)
                ytr = sb5.tile([HID, B], F32)
                nc.sync.dma_start(ytr[:], yt_full[:])
                nc.scalar.activation(
                    yt_aug[0:HID, :],
                    ytr[:],
                    mybir.ActivationFunctionType.Identity,
                    bias=fc1b_sb[:],
                )

                plog = ps5.tile([B, CLS], F32, tag="plog")
                nc.tensor.matmul(plog[:], yt_aug[:], w2_sb[:], start=True, stop=True)
                logit_sb = sb5.tile([B, CLS], F32)
                nc.scalar.copy(logit_sb[:], plog[:])
                # logits read from PSUM on DVE in parallel with the ACT copy
                nc.vector.tensor_copy(merged[:, 0:CLS], plog[:])
                nc.sync.dma_start(logits_d[:], logit_sb[:])

                # softmax over the 16 features (free dim).  No max-subtract:
                # |merged| <= ~6 here, exp() is safely in range, and softmax
                # is shift-invariant so the result matches the reference.
                esb = sb5.tile([B, 4 * CLS], F32)
                ssum = sb5.tile([B, 1], F32)
                nc.scalar.activation(
                    esb[:],
                    merged[:],
                    mybir.ActivationFunctionType.Exp,
                    accum_out=ssum[:],
                )
                rinv = sb5.tile([B, 1], F32)
                nc.vector.reciprocal(rinv[:], ssum[:])

                # softmax result with a ones column appended (becomes the
                # bias row after the transpose)
                smx = sb5.tile([B, 4 * CLS + 1], F32)
                nc.vector.tensor_scalar_mul(smx[:, 0 : 4 * CLS], esb[:], rinv[:])
                nc.vector.tensor_copy(
                    smx[:, 4 * CLS : 4 * CLS + 1], ones_col[0:B, 0:1]
                )

                # x_merge = smx @ cls_w.T + cls_b  (via transposed smx + aug)
                pmt = ps5.tile([4 * CLS + 1, B], F32, tag="pmt")
                nc.tensor.transpose(pmt[:], smx[:], identf[0:B, 0:B])
                mt_aug = sb5.tile([4 * CLS + 1, B], F32)
                nc.scalar.copy(mt_aug[:], pmt[:])

                pxm = ps5.tile([B, CLS], F32, tag="pxm")
                nc.tensor.matmul(pxm[:], mt_aug[:], wc_sb[:], start=True, stop=True)
                xm_sb = sb5.tile([B, CLS], F32)
                nc.scalar.copy(xm_sb[:], pxm[:])
                nc.sync.dma_start(xmerge_d[:], xm_sb[:])

    return nc


_NC_CACHE = None


def _get_nc():
    global _NC_CACHE
    if _NC_CACHE is None:
        _NC_CACHE = _build_nc()
    return _NC_CACHE


def _make_in_maps(inputs):
    np_mm = np.dtype(mybir.dt.np(MM_DT))

    x1 = np.ascontiguousarray(inputs["x1"], dtype=np.float32).reshape(B, C, L)
    x2 = np.ascontiguousarray(inputs["x2"], dtype=np.float32).reshape(B, C, L)
    x3 = np.ascontiguousarray(inputs["x3"], dtype=np.float32).reshape(B, C, L)
    # (B, L, M) concat + transpose, pad L to 3200, relayout to [B, 128, 25, M]
    xcat = np.concatenate([x1, x2, x3], axis=1).transpose(0, 2, 1)
    xpad = np.zeros((B, LC * 128, M), dtype=np_mm)
    xpad[:, :L] = xcat.astype(np_mm)
    xt_host = np.ascontiguousarray(
        xpad.reshape(B, LC, 128, M).transpose(0, 2, 1, 3)
    )

    x11 = np.ascontiguousarray(inputs["x11"], dtype=np.float32)
    x21 = np.ascontiguousarray(inputs["x21"], dtype=np.float32)
    x31 = np.ascontiguousarray(inputs["x31"], dtype=np.float32)
    fc0_w = np.asarray(inputs["fc0_w"], dtype=np.float32)
    fc0_b = np.asarray(inputs["fc0_b"], dtype=np.float32)
    fc1_w = np.asarray(inputs["fc1_w"], dtype=np.float32)
    fc1_b = np.asarray(inputs["fc1_b"], dtype=np.float32)
    fc2_w = np.asarray(inputs["fc2_w"], dtype=np.float32)
    fc2_b = np.asarray(inputs["fc2_b"], dtype=np.float32)
    cls_w = np.asarray(inputs["cls_w"], dtype=np.float32)
    cls_b = np.asarray(inputs["cls_b"], dtype=np.float32)

    # folded fc0 weights (fp32 fold, then cast)
    w0r = fc0_w.reshape(O0, M, M)
    wA = w0r[:, :MA, :].copy()
    wA[:, :, MA:] += w0r[:, MA:, :MA].transpose(0, 2, 1)
    wA_host = np.ascontiguousarray(wA.transpose(1, 2, 0).astype(np_mm))  # [m, n, o]
    w4_host = np.ascontiguousarray(
        w0r[:, MA:, MA:].transpose(1, 2, 0).astype(np_mm)
    )  # [a, b4, o]

    w1t = np.ascontiguousarray(fc1_w.T)  # (1024, 64)
    fc1b = np.ascontiguousarray(fc1_b.reshape(HID, 1))
    w2t = np.ascontiguousarray(
        np.concatenate([fc2_w.T, fc2_b.reshape(1, CLS)], axis=0)
    )
    wct = np.ascontiguousarray(
        np.concatenate([cls_w.T, cls_b.reshape(1, CLS)], axis=0)
    )

    in_maps = []
    for c in range(N_CORES):
        sl = slice(B_LOC * c, B_LOC * (c + 1))
        ol = slice(O0_LOC * c, O0_LOC * (c + 1))
        in_maps.append(
            {
                "xall": np.ascontiguousarray(xt_host[sl]),
                "x11": x11,
                "x21": x21,
                "x31": x31,
                "wA": np.ascontiguousarray(wA_host[:, :, ol]),
                "w4": np.ascontiguousarray(w4_host[:, :, ol]),
                "fc0b": np.ascontiguousarray(fc0_b[ol].reshape(O0_LOC, 1)),
                "w1t": np.ascontiguousarray(w1t[ol]),
                "fc1b": fc1b,
                "w2t": w2t,
                "wct": wct,
            }
        )
    return in_maps


def run(inputs, trace=False, **kwargs):
    nc = _get_nc()
    in_maps = _make_in_maps(inputs)
    res = run_bass_kernel_spmd(nc, in_maps, CORE_IDS, trace=trace, **kwargs)
    out = res.results[0]
    logits = np.asarray(out["logits"], dtype=np.float32)
    x_merge = np.asarray(out["x_merge"], dtype=np.float32)
    return (logits, x_merge), res


def kernel(**inputs):
    (logits, x_merge), _ = run(inputs, trace=False)
    return logits, x_merge


# revision 6
# speedup vs baseline: 1.7541x; 1.6858x over previous
"""Bilinear pooling kernel for 8 Trainium2 NeuronCores (Bass/Tile).

Math (matches the jax reference):
  x = concat([x1, x2, x3], channel) -> (B=64, M=147, L=3136)
  phi_b = x_b @ x_b.T                              (147, 147), symmetric
  phi = sign(phi) * sqrt(|phi| + EPS)              (signed sqrt)
  phi = phi / sqrt(sum(phi^2 + EPS) + 1.0)         (per-batch normalize)
  h = phi_vec @ fc0_w.T + fc0_b                    (64, 1024)
  y = h @ fc1_w.T + fc1_b                          (64, 64)
  logits = y @ fc2_w.T + fc2_b                     (64, 4)
  merged = softmax(concat([logits, x11, x21, x31]))
  x_merge = merged @ cls_w.T + cls_b               (64, 4)
  returns (logits, x_merge)

Key structural tricks:
  * fc0 and fc1 are linear with no nonlinearity between them (dropout is
    identity at inference), so they are fused HOST-SIDE:
    Wc = fc1_w @ fc0_w (64 x 21609, 2.4 MB fp16) is replicated on every
    core, and each core computes y / logits / softmax / x_merge for its
    OWN 8 batches entirely locally.  The only collective is ONE AllGather
    of the per-core (8, 8) output rows at the very end -- this matters
    because the CC stream is blocked by a ~58us comm-init barrier and each
    collective op costs ~11us regardless of size.
  * phi is symmetric: only the top 128x147 block (A) and the 19x19 corner
    are computed; Wc is folded host-side (WcA[m,n] += Wc[n*147+m] for
    n>=128) so no transpose of phi is ever needed and the bilinear pass
    does 147+19 instead of 2*147 matmul columns per l-chunk.
  * all big DMAs are partition-contiguous (host pre-transposes): each
    transfer is 128 descriptors of >=512B instead of thousands of ~280B.

Distribution:
  phase 0: preload fused weights + tail constants (overlaps phase 1)
  phase 1: batch-parallel bilinear+signed-sqrt+normalize (8 batches/core),
           normalized phi written straight into SBUF (no DRAM roundtrip)
  phase 2: fused fc0+fc1: 147+19+1 accumulating chunk matmuls -> y (8, 64)
  phase 3: local tail (fc2, softmax, cls) for the core's 8 batches
  phase 4: AllGather the (8, 8) output rows; all cores write the full
           (64, 4) outputs (harness reads core 0)
"""

import sys

sys.path.insert(0, "/opt/trn_rl_repo")

import numpy as np

import concourse.bass as bass
import concourse.tile as tile
from concourse import masks, mybir
from concourse.bass_utils import run_bass_kernel_spmd
import bass_rust
from bass_rust import ScopedClock

# ---------------------------------------------------------------------------
# Workaround: this toolchain's walrus accepts only ONE semaphore wait per
# instruction, but Tile can attach several.  Split excess waits onto
# same-engine nops placed immediately before the instruction (same engine
# => executed in order, so synchronization semantics are unchanged).
# ---------------------------------------------------------------------------
_MAX_WAITS = 1
_ws_counter = [0]


def _split_excess_waits(obb):
    for bb, insts in list(obb.items()):
        new_list = []
        for inst in insts:
            info = inst.sync_info
            if info is not None and len(info.on_wait) > _MAX_WAITS:
                waits = list(info.on_wait)
                excess = waits[:-_MAX_WAITS]
                keep = waits[-_MAX_WAITS:]
                for i in range(0, len(excess), _MAX_WAITS):
                    _ws_counter[0] += 1
                    nop = mybir.InstNoOp(
                        name=f"WS-{_ws_counter[0]}",
                        sync_info=bass_rust.SyncInfo(
                            on_wait=excess[i : i + _MAX_WAITS], on_update=[]
                        ),
                        bass_nofuse=True,
                        engine=inst.engine,
                    )
                    new_list.append(nop)
                inst.sync_info = bass_rust.SyncInfo(
                    on_wait=keep, on_update=list(info.on_update)
                )
            new_list.append(inst)
        obb[bb] = new_list


_RealTCW = tile.TileClockWait


class _TCWWrapper:
    def __init__(self, *args, **kwargs):
        self._inner = _RealTCW(*args, **kwargs)
        self._obb = (
            args[1] if len(args) > 1 else kwargs["ordered_instructions_by_block"]
        )

    def __getattr__(self, name):
        return getattr(self._inner, name)

    def assign_waits(self, bb_name):
        self._inner.assign_waits(bb_name)
        _split_excess_waits(self._obb)


tile.TileClockWait = _TCWWrapper


def _split_drain_and_barrier(self, tick_clock, wait_clock):
    nc = self.nc
    drain_inst = nc.sync.drain()
    wait_clock.add_sem_waits(
        drain_inst.ins, ScopedClock({None: tick_clock.global_clock})
    )
    info = drain_inst.ins.sync_info
    if info is not None and len(info.on_wait) > _MAX_WAITS:
        waits = list(info.on_wait)
        drain_inst.ins.sync_info = bass_rust.SyncInfo(
            on_wait=waits[:_MAX_WAITS], on_update=list(info.on_update)
        )
        rest = waits[_MAX_WAITS:]
        while rest:
            chunk, rest = rest[:_MAX_WAITS], rest[_MAX_WAITS:]
            nop_inst = nc.sync.nop(nofuse=True, hint="tail_drain_split")
            nop_inst.ins.sync_info = bass_rust.SyncInfo(on_wait=chunk, on_update=[])
    nc.all_engine_barrier()
    assert self.sems is not None
    popped = nc._tile_sem_poison_stack.pop()
    assert popped is self._sem_poison
    nc.clear_and_free_semaphores(list(self.sems.allocated().values()))
    nc.all_engine_barrier()


tile.TileContext._drain_and_barrier = _split_drain_and_barrier

# ---------------------------------------------------------------------------
# Problem constants (hardcoded per the spec)
# ---------------------------------------------------------------------------
N_CORES = 8
CORE_IDS = list(range(N_CORES))
B = 64
B_LOC = B // N_CORES  # 8 batches per core
C = 49
L = 3136  # 56*56
LC = 25  # l-chunks of 128 (3200 rows, last 64 zero-padded)
M = 147  # 3*49 channels
MA = 128  # top block rows
MB = M - MA  # 19 corner rows
MM = M * M  # 21609
O0 = 1024  # fc0 out features
HID = 64  # fc1 out features
CLS = 4
NOUT = 2 * CLS  # logits + x_merge packed per batch row
EPS = 1e-8
# normalizer constant: sum(phi_ss^2 + EPS) + 1.0 == sum|phi| + 2*MM*EPS + 1.0
NORM_C = float(2 * MM * EPS + 1.0)

F32 = mybir.dt.float32
MM_DT = mybir.dt.float16
W_DMA = 37  # n-chunks per WcA weight DMA (4 transfers)


def _build_nc():
    nc = bass.Bass()

    # -- external I/O ------------------------------------------------------
    # x arrives host-side concatenated, zero-padded to 3200 l-rows, and
    # pre-transposed to [b][p=128][lc=25][m=147] so each per-batch DMA is
    # 128 descriptors x 7350 B.
    xall_d = nc.dram_tensor("xall", [B_LOC, 128, LC, M], MM_DT, kind="ExternalInput")
    # per-core slices of x11/x21/x31 (this core's 8 batches)
    x11_d = nc.dram_tensor("x11", [B_LOC, CLS], F32, kind="ExternalInput")
    x21_d = nc.dram_tensor("x21", [B_LOC, CLS], F32, kind="ExternalInput")
    x31_d = nc.dram_tensor("x31", [B_LOC, CLS], F32, kind="ExternalInput")
    # fused fc1@fc0 weights, symmetric-folded:
    #   wA[m, n, y] = Wc[y, m*147+n] (+ Wc[y, n*147+m] for n >= 128)
    #   w4[a, b4, y] = Wc[y, (128+a)*147 + 128+b4]
    wA_d = nc.dram_tensor("wA", [MA, M, HID], MM_DT, kind="ExternalInput")
    w4_d = nc.dram_tensor("w4", [MB, MB, HID], MM_DT, kind="ExternalInput")
    bc1_d = nc.dram_tensor("bc1", [1, HID], F32, kind="ExternalInput")
    w2t_d = nc.dram_tensor("w2t", [HID + 1, CLS], F32, kind="ExternalInput")
    wct_d = nc.dram_tensor("wct", [4 * CLS + 1, CLS], F32, kind="ExternalInput")
    logits_d = nc.dram_tensor("logits", [B, CLS], F32, kind="ExternalOutput")
    xmerge_d = nc.dram_tensor("x_merge", [B, CLS], F32, kind="ExternalOutput")

    with tile.TileContext(nc) as tc:
        with tc.tile_pool(name="dram", bufs=1, space="DRAM") as dram, tc.tile_pool(
            name="const", bufs=1
        ) as const:
            # -- collective buffers ------------------------------------
            outb = dram.tile([B_LOC, NOUT], F32)
            outg = dram.tile([B, NOUT], F32, addr_space="Shared")

            # -- constants ----------------------------------------------
            identf = const.tile([128, 128], F32)
            masks.make_identity(nc, identf[:])
            ones_col = const.tile([128, 128], F32)
            nc.gpsimd.memset(ones_col[:], 1.0)
            eps_col = const.tile([128, 1], F32)
            nc.gpsimd.memset(eps_col[:], EPS)
            normc_col = const.tile([128, 1], F32)
            nc.gpsimd.memset(normc_col[:], NORM_C)

            # normalized phi blocks, SBUF-resident across phases 1-2
            pallA = const.tile([MA, B_LOC, M], MM_DT)  # [p=m, bl, n]
            pallB = const.tile([MB, B_LOC, MB], MM_DT)  # [p=a, bl, b4]

            # ===========================================================
            # phase 1 + 0: per-batch bilinear pipeline; weight preloads
            # are issued after the first xt DMAs so x gets DMA priority
            # ===========================================================
            with tc.tile_pool(name="xt", bufs=3) as xt_pool, tc.tile_pool(
                name="p1sb", bufs=2
            ) as sb, tc.tile_pool(
                name="p1ps", bufs=2, space="PSUM"
            ) as ps, nc.named_scope("p1_bilinear"):

                def p1_mains(b):
                    xt = xt_pool.tile([128, LC, M], MM_DT, tag="xt")
                    nc.sync.dma_start(xt[:], xall_d[b])
                    # A block: phi[0:128, 0:147]; corner: phi[128:147, 128:147]
                    pA = ps.tile([MA, M], F32, tag="pA", bufs=3)
                    pB2 = ps.tile([MB, MB], F32, tag="pB2", bufs=3)
                    for lc in range(LC):
                        nc.tensor.matmul(
                            pA[:],
                            xt[:, lc, 0:MA],
                            xt[:, lc, :],
                            start=(lc == 0),
                            stop=(lc == LC - 1),
                        )
                    for lc in range(LC):
                        nc.tensor.matmul(
                            pB2[:],
                            xt[:, lc, MA:M],
                            xt[:, lc, MA:M],
                            start=(lc == 0),
                            stop=(lc == LC - 1),
                        )
                    return pA, pB2

                def p1_norm(b, pA, pB2):
                    # signed sqrt pieces + |phi| row sums (accumulated on ACT)
                    sgnA = sb.tile([MA, M], F32, tag="sgnA")
                    absA = sb.tile([MA, M], F32, tag="absA")
                    rsA = sb.tile([MA, 1], F32, tag="rsA")
                    nc.scalar.activation(
                        sgnA[:], pA[:], mybir.ActivationFunctionType.Sign
                    )
                    nc.scalar.activation(
                        absA[:],
                        pA[:],
                        mybir.ActivationFunctionType.Abs,
                        accum_out=rsA[:],
                    )
                    sgnB2 = sb.tile([MB, MB], F32, tag="sgnB2")
                    absB2 = sb.tile([MB, MB], F32, tag="absB2")
                    rsB2 = sb.tile([MB, 1], F32, tag="rsB2")
                    nc.scalar.activation(
                        sgnB2[:], pB2[:], mybir.ActivationFunctionType.Sign
                    )
                    nc.scalar.activation(
                        absB2[:],
                        pB2[:],
                        mybir.ActivationFunctionType.Abs,
                        accum_out=rsB2[:],
                    )
                    # S2 columns counted twice (symmetric image lives in S3)
                    rsA2 = sb.tile([MA, 1], F32, tag="rsA2")
                    nc.vector.reduce_sum(
                        rsA2[:], absA[:, MA:M], axis=mybir.AxisListType.X
                    )
                    rsT = sb.tile([MA, 1], F32, tag="rsT")
                    nc.vector.tensor_add(rsT[:], rsA[:], rsA2[:])

                    # cross-partition sum + broadcast in one accumulation
                    # group: bc[m] = sum_k ones[k, m] * rs[k]
                    bc = ps.tile([128, 1], F32, tag="bc")
                    nc.tensor.matmul(
                        bc[:], ones_col[:, :], rsT[:], start=True, stop=False
                    )
                    nc.tensor.matmul(
                        bc[:], ones_col[0:MB, :], rsB2[:], start=False, stop=True
                    )

                    # ss = sign * sqrt(|phi| + EPS)
                    sqA = sb.tile([MA, M], F32, tag="sqA")
                    nc.scalar.activation(
                        sqA[:],
                        absA[:],
                        mybir.ActivationFunctionType.Sqrt,
                        bias=eps_col[:],
                    )
                    ssA = sb.tile([MA, M], F32, tag="ssA")
                    nc.vector.tensor_mul(ssA[:], sqA[:], sgnA[:])
                    sqB2 = sb.tile([MB, MB], F32, tag="sqB2")
                    nc.scalar.activation(
                        sqB2[:],
                        absB2[:],
                        mybir.ActivationFunctionType.Sqrt,
                        bias=eps_col[0:MB],
                    )
                    ssB2 = sb.tile([MB, MB], F32, tag="ssB2")
                    nc.vector.tensor_mul(ssB2[:], sqB2[:], sgnB2[:])

                    # scale = 1 / sqrt(total + NORM_C)
                    inv = sb.tile([128, 1], F32, tag="inv")
                    nc.scalar.activation(
                        inv[:],
                        bc[:],
                        mybir.ActivationFunctionType.Sqrt,
                        bias=normc_col[:],
                    )
                    scl = sb.tile([128, 1], F32, tag="scl")
                    nc.vector.reciprocal(scl[:], inv[:])

                    # normalized phi straight into the SBUF-resident blocks
                    nc.vector.tensor_scalar_mul(
                        pallA[:, b, :], ssA[:], scl[0:MA]
                    )
                    nc.vector.tensor_scalar_mul(
                        pallB[:, b, :], ssB2[:], scl[0:MB]
                    )

                # 1-batch software pipeline: batch b's norm chain is issued
                # after batch b+1's matmuls so the PE stream never stalls
                prev = None
                for b in range(B_LOC):
                    cur = (b, *p1_mains(b))
                    if b == 0:
                        # weight preloads issued behind the first xt DMA
                        w_sb = const.tile([MA, M, HID], MM_DT)
                        for wd in range(0, M, W_DMA):
                            wn = min(W_DMA, M - wd)
                            nc.scalar.dma_start(
                                w_sb[:, wd : wd + wn, :],
                                wA_d[:, wd : wd + wn, :],
                            )
                        w4_sb = const.tile([MB, MB, HID], MM_DT)
                        nc.scalar.dma_start(w4_sb[:], w4_d[:])
                        bc1_sb = const.tile([1, HID], F32)
                        nc.scalar.dma_start(bc1_sb[:], bc1_d[:])
                        w2_sb = const.tile([HID + 1, CLS], F32)
                        nc.scalar.dma_start(w2_sb[:], w2t_d[:])
                        wc_sb = const.tile([4 * CLS + 1, CLS], F32)
                        nc.scalar.dma_start(wc_sb[:], wct_d[:])
                        xm1_sb = const.tile([B_LOC, CLS], F32)
                        nc.scalar.dma_start(xm1_sb[:], x11_d[:])
                        xm2_sb = const.tile([B_LOC, CLS], F32)
                        nc.scalar.dma_start(xm2_sb[:], x21_d[:])
                        xm3_sb = const.tile([B_LOC, CLS], F32)
                        nc.scalar.dma_start(xm3_sb[:], x31_d[:])
                        # pre-staged tail tiles (written once, reused)
                        yt_aug = const.tile([HID + 1, B_LOC], F32)
                        nc.vector.tensor_copy(
                            yt_aug[HID : HID + 1, :], ones_col[0:1, 0:B_LOC]
                        )
                        merged = const.tile([B_LOC, 4 * CLS], F32)
                        nc.vector.tensor_copy(merged[:, CLS : 2 * CLS], xm1_sb[:])
                        nc.vector.tensor_copy(
                            merged[:, 2 * CLS : 3 * CLS], xm2_sb[:]
                        )
                        nc.vector.tensor_copy(
                            merged[:, 3 * CLS : 4 * CLS], xm3_sb[:]
                        )
                    if prev is not None:
                        p1_norm(*prev)
                    prev = cur
                p1_norm(*prev)

            # ===========================================================
            # phase 2: fused fc0+fc1 -> y (8, 64), bias folded in as an
            # extra rank-1 chunk with a ones stationary
            # ===========================================================
            with tc.tile_pool(name="p3sb", bufs=1) as sb3, tc.tile_pool(
                name="p3ps", bufs=1, space="PSUM"
            ) as ps3, nc.named_scope("p2_fc01"):
                py = ps3.tile([B_LOC, HID], F32, tag="py")
                for n in range(M):
                    nc.tensor.matmul(
                        py[:],
                        pallA[:, :, n],
                        w_sb[:, n, :],
                        start=(n == 0),
                        stop=False,
                    )
                for b4 in range(MB):
                    nc.tensor.matmul(
                        py[:],
                        pallB[:, :, b4],
                        w4_sb[:, b4, :],
                        start=False,
                        stop=False,
                    )
                nc.tensor.matmul(
                    py[:],
                    ones_col[0:1, 0:B_LOC],
                    bc1_sb[:],
                    start=False,
                    stop=True,
                )

                # y^T via PE transpose, ones row pre-staged for the fc2 bias
                y_sb = sb3.tile([B_LOC, HID], F32)
                nc.scalar.copy(y_sb[:], py[:])
                pyt = ps3.tile([HID, B_LOC], F32, tag="pyt")
                nc.tensor.transpose(pyt[:], y_sb[:], identf[0:B_LOC, 0:B_LOC])
                nc.scalar.copy(yt_aug[0:HID, :], pyt[:])

            # ===========================================================
            # phase 3: local tail for this core's 8 batches
            # ===========================================================
            with tc.tile_pool(name="p5sb", bufs=1) as sb5, tc.tile_pool(
                name="p5ps", bufs=1, space="PSUM"
            ) as ps5, nc.named_scope("p3_tail"):
                plog = ps5.tile([B_LOC, CLS], F32, tag="plog")
                nc.tensor.matmul(plog[:], yt_aug[:], w2_sb[:], start=True, stop=True)
                out_sb = sb5.tile([B_LOC, NOUT], F32)
                nc.scalar.copy(out_sb[:, 0:CLS], plog[:])
                # logits read from PSUM on DVE in parallel with the ACT copy
                nc.vector.tensor_copy(merged[:, 0:CLS], plog[:])

                # softmax over the 16 features (free dim).  No max-subtract:
                # |merged| <= ~6 here, exp() is safely in range, and softmax
                # is shift-invariant so the result matches the reference.
                esb = sb5.tile([B_LOC, 4 * CLS], F32)
                ssum = sb5.tile([B_LOC, 1], F32)
                nc.scalar.activation(
                    esb[:],
                    merged[:],
                    mybir.ActivationFunctionType.Exp,
                    accum_out=ssum[:],
                )
                rinv = sb5.tile([B_LOC, 1], F32)
                nc.vector.reciprocal(rinv[:], ssum[:])

                # softmax result with a ones column appended (becomes the
                # bias row after the transpose)
                smx = sb5.tile([B_LOC, 4 * CLS + 1], F32)
                nc.vector.tensor_scalar_mul(smx[:, 0 : 4 * CLS], esb[:], rinv[:])
                nc.vector.tensor_copy(
                    smx[:, 4 * CLS : 4 * CLS + 1], ones_col[0:B_LOC, 0:1]
                )

                # x_merge = smx @ cls_w.T + cls_b  (via transposed smx + aug)
                pmt = ps5.tile([4 * CLS + 1, B_LOC], F32, tag="pmt")
                nc.tensor.transpose(pmt[:], smx[:], identf[0:B_LOC, 0:B_LOC])
                mt_aug = sb5.tile([4 * CLS + 1, B_LOC], F32)
                nc.scalar.copy(mt_aug[:], pmt[:])

                pxm = ps5.tile([B_LOC, CLS], F32, tag="pxm")
                nc.tensor.matmul(pxm[:], mt_aug[:], wc_sb[:], start=True, stop=True)
                nc.scalar.copy(out_sb[:, CLS:NOUT], pxm[:])
                nc.sync.dma_start(outb[:], out_sb[:])

            # ===========================================================
            # phase 4: gather all cores' output rows; write full outputs
            # ===========================================================
            with tc.tile_pool(name="p4sb", bufs=1) as sb4, nc.named_scope("p4_out"):
                nc.gpsimd.collective_compute(
                    "AllGather",
                    mybir.AluOpType.bypass,
                    replica_groups=[CORE_IDS],
                    ins=[outb.opt()],
                    outs=[outg.opt()],
                )
                land = sb4.tile([B, NOUT], F32)
                nc.sync.dma_start(land[:], outg[:])
                nc.sync.dma_start(logits_d[:], land[:, 0:CLS])
                nc.sync.dma_start(xmerge_d[:], land[:, CLS:NOUT])

    return nc


_NC_CACHE = None


def _get_nc():
    global _NC_CACHE
    if _NC_CACHE is None:
        _NC_CACHE = _build_nc()
    return _NC_CACHE


def _make_in_maps(inputs):
    np_mm = np.dtype(mybir.dt.np(MM_DT))

    x1 = np.ascontiguousarray(inputs["x1"], dtype=np.float32).reshape(B, C, L)
    x2 = np.ascontiguousarray(inputs["x2"], dtype=np.float32).reshape(B, C, L)
    x3 = np.ascontiguousarray(inputs["x3"], dtype=np.float32).reshape(B, C, L)
    # (B, L, M) concat + transpose, pad L to 3200, relayout to [B, 128, 25, M]
    xcat = np.concatenate([x1, x2, x3], axis=1).transpose(0, 2, 1)
    xpad = np.zeros((B, LC * 128, M), dtype=np_mm)
    xpad[:, :L] = xcat.astype(np_mm)
    xt_host = np.ascontiguousarray(
        xpad.reshape(B, LC, 128, M).transpose(0, 2, 1, 3)
    )

    x11 = np.ascontiguousarray(inputs["x11"], dtype=np.float32)
    x21 = np.ascontiguousarray(inputs["x21"], dtype=np.float32)
    x31 = np.ascontiguousarray(inputs["x31"], dtype=np.float32)
    fc0_w = np.asarray(inputs["fc0_w"], dtype=np.float32)
    fc0_b = np.asarray(inputs["fc0_b"], dtype=np.float32)
    fc1_w = np.asarray(inputs["fc1_w"], dtype=np.float32)
    fc1_b = np.asarray(inputs["fc1_b"], dtype=np.float32)
    fc2_w = np.asarray(inputs["fc2_w"], dtype=np.float32)
    fc2_b = np.asarray(inputs["fc2_b"], dtype=np.float32)
    cls_w = np.asarray(inputs["cls_w"], dtype=np.float32)
    cls_b = np.asarray(inputs["cls_b"], dtype=np.float32)

    # fuse fc0+fc1 (both linear; dropout is identity at inference), then
    # apply the symmetric fold (fp32 fold, cast to fp16 at the end)
    Wc = fc1_w @ fc0_w  # (64, 21609)
    bc1 = (fc1_w @ fc0_b + fc1_b).reshape(1, HID)
    Wcr = Wc.reshape(HID, M, M)
    WcA = Wcr[:, :MA, :].copy()
    WcA[:, :, MA:] += Wcr[:, MA:, :MA].transpose(0, 2, 1)
    wA_host = np.ascontiguousarray(WcA.transpose(1, 2, 0).astype(np_mm))  # [m, n, y]
    w4_host = np.ascontiguousarray(
        Wcr[:, MA:, MA:].transpose(1, 2, 0).astype(np_mm)
    )  # [a, b4, y]

    w2t = np.ascontiguousarray(
        np.concatenate([fc2_w.T, fc2_b.reshape(1, CLS)], axis=0)
    )
    wct = np.ascontiguousarray(
        np.concatenate([cls_w.T, cls_b.reshape(1, CLS)], axis=0)
    )

    in_maps = []
    for c in range(N_CORES):
        sl = slice(B_LOC * c, B_LOC * (c + 1))
        in_maps.append(
            {
                "xall": np.ascontiguousarray(xt_host[sl]),
                "x11": np.ascontiguousarray(x11[sl]),
                "x21": np.ascontiguousarray(x21[sl]),
                "x31": np.ascontiguousarray(x31[sl]),
                "wA": wA_host,
                "w4": w4_host,
                "bc1": np.ascontiguousarray(bc1),
                "w2t": w2t,
                "wct": wct,
            }
        )
    return in_maps


def run(inputs, trace=False, **kwargs):
    nc = _get_nc()
    in_maps = _make_in_maps(inputs)
    res = run_bass_kernel_spmd(nc, in_maps, CORE_IDS, trace=trace, **kwargs)
    out = res.results[0]
    logits = np.asarray(out["logits"], dtype=np.float32)
    x_merge = np.asarray(out["x_merge"], dtype=np.float32)
    return (logits, x_merge), res


def kernel(**inputs):
    (logits, x_merge), _ = run(inputs, trace=False)
    return logits, x_merge


# revision 11
# speedup vs baseline: 2.8093x; 1.6015x over previous
"""Bilinear pooling kernel for 8 Trainium2 NeuronCores (Bass/Tile).

Math (matches the jax reference):
  x = concat([x1, x2, x3], channel) -> (B=64, M=147, L=3136)
  phi_b = x_b @ x_b.T                              (147, 147), symmetric
  phi = sign(phi) * sqrt(|phi| + EPS)              (signed sqrt)
  phi = phi / sqrt(sum(phi^2 + EPS) + 1.0)         (per-batch normalize)
  h = phi_vec @ fc0_w.T + fc0_b                    (64, 1024)
  y = h @ fc1_w.T + fc1_b                          (64, 64)
  logits = y @ fc2_w.T + fc2_b                     (64, 4)
  merged = softmax(concat([logits, x11, x21, x31]))
  x_merge = merged @ cls_w.T + cls_b               (64, 4)
  returns (logits, x_merge)

Key structural tricks:
  * fc0 and fc1 are linear with no nonlinearity between them (dropout is
    identity at inference), so they are fused HOST-SIDE:
    Wc = fc1_w @ fc0_w (64 x 21609, 2.4 MB fp16) is replicated on every
    core, and each core computes y / logits / softmax / x_merge for its
    OWN 8 batches entirely locally.  The only collective is ONE AllGather
    of the per-core (8, 8) output rows at the very end -- this matters
    because the CC stream is blocked by a ~58us comm-init barrier and each
    collective op costs ~11us regardless of size.
  * phi is symmetric: only the top 128x147 block (A) and the 19x19 corner
    are computed; Wc is folded host-side (WcA[m,n] += Wc[n*147+m] for
    n>=128) so no transpose of phi is ever needed and the bilinear pass
    does 147+19 instead of 2*147 matmul columns per l-chunk.
  * all big DMAs are partition-contiguous (host pre-transposes): each
    transfer is 128 descriptors of >=512B instead of thousands of ~280B.

Distribution:
  phase 0: preload fused weights + tail constants (overlaps phase 1)
  phase 1: batch-parallel bilinear+signed-sqrt+normalize (8 batches/core),
           normalized phi written straight into SBUF (no DRAM roundtrip)
  phase 2: fused fc0+fc1: 147+19+1 accumulating chunk matmuls -> y (8, 64)
  phase 3: local tail (fc2, softmax, cls) for the core's 8 batches
  phase 4: AllGather the (8, 8) output rows; all cores write the full
           (64, 4) outputs (harness reads core 0)
"""

import sys

sys.path.insert(0, "/opt/trn_rl_repo")

import numpy as np

import concourse.bass as bass
import concourse.tile as tile
from concourse import masks, mybir
from concourse.bass_utils import run_bass_kernel_spmd
import bass_rust
from bass_rust import ScopedClock

# ---------------------------------------------------------------------------
# Workaround: this toolchain's walrus accepts only ONE semaphore wait per
# instruction, but Tile can attach several.  Split excess waits onto
# same-engine nops placed immediately before the instruction (same engine
# => executed in order, so synchronization semantics are unchanged).
# ---------------------------------------------------------------------------
_MAX_WAITS = 1
_ws_counter = [0]


def _split_excess_waits(obb):
    for bb, insts in list(obb.items()):
        new_list = []
        for inst in insts:
            info = inst.sync_info
            if info is not None and len(info.on_wait) > _MAX_WAITS:
                waits = list(info.on_wait)
                excess = waits[:-_MAX_WAITS]
                keep = waits[-_MAX_WAITS:]
                for i in range(0, len(excess), _MAX_WAITS):
                    _ws_counter[0] += 1
                    nop = mybir.InstNoOp(
                        name=f"WS-{_ws_counter[0]}",
                        sync_info=bass_rust.SyncInfo(
                            on_wait=excess[i : i + _MAX_WAITS], on_update=[]
                        ),
                        bass_nofuse=True,
                        engine=inst.engine,
                    )
                    new_list.append(nop)
                inst.sync_info = bass_rust.SyncInfo(
                    on_wait=keep, on_update=list(info.on_update)
                )
            new_list.append(inst)
        obb[bb] = new_list


_RealTCW = tile.TileClockWait


class _TCWWrapper:
    def __init__(self, *args, **kwargs):
        self._inner = _RealTCW(*args, **kwargs)
        self._obb = (
            args[1] if len(args) > 1 else kwargs["ordered_instructions_by_block"]
        )

    def __getattr__(self, name):
        return getattr(self._inner, name)

    def assign_waits(self, bb_name):
        self._inner.assign_waits(bb_name)
        _split_excess_waits(self._obb)


tile.TileClockWait = _TCWWrapper


def _split_drain_and_barrier(self, tick_clock, wait_clock):
    nc = self.nc
    drain_inst = nc.sync.drain()
    wait_clock.add_sem_waits(
        drain_inst.ins, ScopedClock({None: tick_clock.global_clock})
    )
    info = drain_inst.ins.sync_info
    if info is not None and len(info.on_wait) > _MAX_WAITS:
        waits = list(info.on_wait)
        drain_inst.ins.sync_info = bass_rust.SyncInfo(
            on_wait=waits[:_MAX_WAITS], on_update=list(info.on_update)
        )
        rest = waits[_MAX_WAITS:]
        while rest:
            chunk, rest = rest[:_MAX_WAITS], rest[_MAX_WAITS:]
            nop_inst = nc.sync.nop(nofuse=True, hint="tail_drain_split")
            nop_inst.ins.sync_info = bass_rust.SyncInfo(on_wait=chunk, on_update=[])
    nc.all_engine_barrier()
    assert self.sems is not None
    popped = nc._tile_sem_poison_stack.pop()
    assert popped is self._sem_poison
    nc.clear_and_free_semaphores(list(self.sems.allocated().values()))
    nc.all_engine_barrier()


tile.TileContext._drain_and_barrier = _split_drain_and_barrier

# ---------------------------------------------------------------------------
# Problem constants (hardcoded per the spec)
# ---------------------------------------------------------------------------
N_CORES = 8
CORE_IDS = list(range(N_CORES))
B = 64
B_LOC = B // N_CORES  # 8 batches per core
C = 49
L = 3136  # 56*56
LC = 25  # l-chunks of 128 (3200 rows, last 64 zero-padded)
M = 147  # 3*49 channels
MA = 128  # top block rows
MB = M - MA  # 19 corner rows
MM = M * M  # 21609
O0 = 1024  # fc0 out features
HID = 64  # fc1 out features
CLS = 4
NOUT = 2 * CLS  # logits + x_merge packed per batch row
EPS = 1e-8
# normalizer constant: sum(phi_ss^2 + EPS) + 1.0 == sum|phi| + 2*MM*EPS + 1.0
NORM_C = float(2 * MM * EPS + 1.0)

F32 = mybir.dt.float32
MM_DT = mybir.dt.float16
W_DMA = 37  # n-chunks per WcA weight DMA (4 transfers)


def _build_nc():
    nc = bass.Bass()

    # -- external I/O ------------------------------------------------------
    # x arrives host-side concatenated, zero-padded to 3200 l-rows, and
    # pre-transposed to [b][p=128][lc=25][m=147] so each per-batch DMA is
    # 128 descriptors x 7350 B.
    xall_d = nc.dram_tensor("xall", [B_LOC, 128, LC, M], MM_DT, kind="ExternalInput")
    # per-core slices of x11/x21/x31 (this core's 8 batches)
    x11_d = nc.dram_tensor("x11", [B_LOC, CLS], F32, kind="ExternalInput")
    x21_d = nc.dram_tensor("x21", [B_LOC, CLS], F32, kind="ExternalInput")
    x31_d = nc.dram_tensor("x31", [B_LOC, CLS], F32, kind="ExternalInput")
    # fused fc1@fc0 weights, symmetric-folded:
    #   wA[m, n, y] = Wc[y, m*147+n] (+ Wc[y, n*147+m] for n >= 128)
    #   w4[a, b4, y] = Wc[y, (128+a)*147 + 128+b4]
    wA_d = nc.dram_tensor("wA", [MA, M, HID], MM_DT, kind="ExternalInput")
    w4_d = nc.dram_tensor("w4", [MB, MB, HID], MM_DT, kind="ExternalInput")
    bc1_d = nc.dram_tensor("bc1", [1, HID], F32, kind="ExternalInput")
    w2t_d = nc.dram_tensor("w2t", [HID + 1, CLS], F32, kind="ExternalInput")
    wct_d = nc.dram_tensor("wct", [4 * CLS + 1, CLS], F32, kind="ExternalInput")
    # each core writes only its own 8 batches; the host concatenates
    logits_d = nc.dram_tensor("logits", [B_LOC, CLS], F32, kind="ExternalOutput")
    xmerge_d = nc.dram_tensor("x_merge", [B_LOC, CLS], F32, kind="ExternalOutput")

    with tile.TileContext(nc) as tc:
        with tc.tile_pool(name="const", bufs=1) as const:
            # -- constants ----------------------------------------------
            identf = const.tile([128, 128], F32)
            masks.make_identity(nc, identf[:])
            ones_col = const.tile([128, 128], F32)
            nc.gpsimd.memset(ones_col[:], 1.0)
            eps_col = const.tile([128, 1], F32)
            nc.gpsimd.memset(eps_col[:], EPS)
            normc_col = const.tile([128, 1], F32)
            nc.gpsimd.memset(normc_col[:], NORM_C)

            # normalized phi blocks, SBUF-resident across phases 1-2
            pallA = const.tile([MA, B_LOC, M], MM_DT)  # [p=m, bl, n]
            pallB = const.tile([MB, B_LOC, MB], MM_DT)  # [p=a, bl, b4]

            # ===========================================================
            # phase 1 + 0: per-batch bilinear pipeline; weight preloads
            # are issued after the first xt DMAs so x gets DMA priority
            # ===========================================================
            with tc.tile_pool(name="xt", bufs=5) as xt_pool, tc.tile_pool(
                name="p1sb", bufs=2
            ) as sb, tc.tile_pool(
                name="p1ps", bufs=2, space="PSUM"
            ) as ps, nc.named_scope("p1_bilinear"):

                def p1_mains(b):
                    xt = xt_pool.tile([128, LC, M], MM_DT, tag="xt")
                    nc.sync.dma_start(xt[:], xall_d[b])
                    # A block: phi[0:128, 0:147]; corner: phi[128:147, 128:147]
                    pA = ps.tile([MA, M], F32, tag="pA", bufs=3)
                    pB2 = ps.tile([MB, MB], F32, tag="pB2", bufs=3)
                    for lc in range(LC):
                        nc.tensor.matmul(
                            pA[:],
                            xt[:, lc, 0:MA],
                            xt[:, lc, :],
                            start=(lc == 0),
                            stop=(lc == LC - 1),
                        )
                    for lc in range(LC):
                        nc.tensor.matmul(
                            pB2[:],
                            xt[:, lc, MA:M],
                            xt[:, lc, MA:M],
                            start=(lc == 0),
                            stop=(lc == LC - 1),
                        )
                    return pA, pB2

                def p1_norm(b, pA, pB2):
                    # signed sqrt pieces + |phi| row sums (accumulated on ACT)
                    sgnA = sb.tile([MA, M], F32, tag="sgnA")
                    absA = sb.tile([MA, M], F32, tag="absA")
                    rsA = sb.tile([MA, 1], F32, tag="rsA")
                    nc.scalar.activation(
                        sgnA[:], pA[:], mybir.ActivationFunctionType.Sign
                    )
                    nc.scalar.activation(
                        absA[:],
                        pA[:],
                        mybir.ActivationFunctionType.Abs,
                        accum_out=rsA[:],
                    )
                    sgnB2 = sb.tile([MB, MB], F32, tag="sgnB2")
                    absB2 = sb.tile([MB, MB], F32, tag="absB2")
                    rsB2 = sb.tile([MB, 1], F32, tag="rsB2")
                    nc.scalar.activation(
                        sgnB2[:], pB2[:], mybir.ActivationFunctionType.Sign
                    )
                    nc.scalar.activation(
                        absB2[:],
                        pB2[:],
                        mybir.ActivationFunctionType.Abs,
                        accum_out=rsB2[:],
                    )
                    # S2 columns counted twice (symmetric image lives in S3)
                    rsA2 = sb.tile([MA, 1], F32, tag="rsA2")
                    nc.vector.reduce_sum(
                        rsA2[:], absA[:, MA:M], axis=mybir.AxisListType.X
                    )
                    rsT = sb.tile([MA, 1], F32, tag="rsT")
                    nc.vector.tensor_add(rsT[:], rsA[:], rsA2[:])

                    # cross-partition sum + broadcast in one accumulation
                    # group: bc[m] = sum_k ones[k, m] * rs[k]
                    bc = ps.tile([128, 1], F32, tag="bc")
                    nc.tensor.matmul(
                        bc[:], ones_col[:, :], rsT[:], start=True, stop=False
                    )
                    nc.tensor.matmul(
                        bc[:], ones_col[0:MB, :], rsB2[:], start=False, stop=True
                    )

                    # ss = sign * sqrt(|phi| + EPS)
                    sqA = sb.tile([MA, M], F32, tag="sqA")
                    nc.scalar.activation(
                        sqA[:],
                        absA[:],
                        mybir.ActivationFunctionType.Sqrt,
                        bias=eps_col[:],
                    )
                    ssA = sb.tile([MA, M], F32, tag="ssA")
                    nc.vector.tensor_mul(ssA[:], sqA[:], sgnA[:])
                    sqB2 = sb.tile([MB, MB], F32, tag="sqB2")
                    nc.scalar.activation(
                        sqB2[:],
                        absB2[:],
                        mybir.ActivationFunctionType.Sqrt,
                        bias=eps_col[0:MB],
                    )
                    ssB2 = sb.tile([MB, MB], F32, tag="ssB2")
                    nc.vector.tensor_mul(ssB2[:], sqB2[:], sgnB2[:])

                    # scale = 1 / sqrt(total + NORM_C)
                    inv = sb.tile([128, 1], F32, tag="inv")
                    nc.scalar.activation(
                        inv[:],
                        bc[:],
                        mybir.ActivationFunctionType.Sqrt,
                        bias=normc_col[:],
                    )
                    scl = sb.tile([128, 1], F32, tag="scl")
                    nc.vector.reciprocal(scl[:], inv[:])

                    # normalized phi straight into the SBUF-resident blocks
                    nc.vector.tensor_scalar_mul(
                        pallA[:, b, :], ssA[:], scl[0:MA]
                    )
                    nc.vector.tensor_scalar_mul(
                        pallB[:, b, :], ssB2[:], scl[0:MB]
                    )

                # 1-batch software pipeline: batch b's norm chain is issued
                # after batch b+1's matmuls so the PE stream never stalls
                prev = None
                for b in range(B_LOC):
                    cur = (b, *p1_mains(b))
                    if b == 0:
                        # weight preloads issued behind the first xt DMA
                        w_sb = const.tile([MA, M, HID], MM_DT)
                        for wd in range(0, M, W_DMA):
                            wn = min(W_DMA, M - wd)
                            nc.scalar.dma_start(
                                w_sb[:, wd : wd + wn, :],
                                wA_d[:, wd : wd + wn, :],
                            )
                        w4_sb = const.tile([MB, MB, HID], MM_DT)
                        nc.scalar.dma_start(w4_sb[:], w4_d[:])
                        bc1_sb = const.tile([1, HID], F32)
                        nc.scalar.dma_start(bc1_sb[:], bc1_d[:])
                        w2_sb = const.tile([HID + 1, CLS], F32)
                        nc.scalar.dma_start(w2_sb[:], w2t_d[:])
                        wc_sb = const.tile([4 * CLS + 1, CLS], F32)
                        nc.scalar.dma_start(wc_sb[:], wct_d[:])
                        xm1_sb = const.tile([B_LOC, CLS], F32)
                        nc.scalar.dma_start(xm1_sb[:], x11_d[:])
                        xm2_sb = const.tile([B_LOC, CLS], F32)
                        nc.scalar.dma_start(xm2_sb[:], x21_d[:])
                        xm3_sb = const.tile([B_LOC, CLS], F32)
                        nc.scalar.dma_start(xm3_sb[:], x31_d[:])
                        # pre-staged tail tiles (written once, reused)
                        yt_aug = const.tile([HID + 1, B_LOC], F32)
                        nc.vector.tensor_copy(
                            yt_aug[HID : HID + 1, :], ones_col[0:1, 0:B_LOC]
                        )
                        merged = const.tile([B_LOC, 4 * CLS], F32)
                        nc.vector.tensor_copy(merged[:, CLS : 2 * CLS], xm1_sb[:])
                        nc.vector.tensor_copy(
                            merged[:, 2 * CLS : 3 * CLS], xm2_sb[:]
                        )
                        nc.vector.tensor_copy(
                            merged[:, 3 * CLS : 4 * CLS], xm3_sb[:]
                        )
                    if prev is not None:
                        p1_norm(*prev)
                    prev = cur
                p1_norm(*prev)

            # ===========================================================
            # phase 2: fused fc0+fc1 -> y (8, 64), bias folded in as an
            # extra rank-1 chunk with a ones stationary
            # ===========================================================
            with tc.tile_pool(name="p3sb", bufs=1) as sb3, tc.tile_pool(
                name="p3ps", bufs=1, space="PSUM"
            ) as ps3, nc.named_scope("p2_fc01"):
                py = ps3.tile([B_LOC, HID], F32, tag="py")
                for n in range(M):
                    nc.tensor.matmul(
                        py[:],
                        pallA[:, :, n],
                        w_sb[:, n, :],
                        start=(n == 0),
                        stop=False,
                    )
                for b4 in range(MB):
                    nc.tensor.matmul(
                        py[:],
                        pallB[:, :, b4],
                        w4_sb[:, b4, :],
                        start=False,
                        stop=False,
                    )
                nc.tensor.matmul(
                    py[:],
                    ones_col[0:1, 0:B_LOC],
                    bc1_sb[:],
                    start=False,
                    stop=True,
                )

                # y^T via PE transpose, ones row pre-staged for the fc2 bias
                y_sb = sb3.tile([B_LOC, HID], F32)
                nc.scalar.copy(y_sb[:], py[:])
                pyt = ps3.tile([HID, B_LOC], F32, tag="pyt")
                nc.tensor.transpose(pyt[:], y_sb[:], identf[0:B_LOC, 0:B_LOC])
                nc.scalar.copy(yt_aug[0:HID, :], pyt[:])

            # ===========================================================
            # phase 3: local tail for this core's 8 batches
            # ===========================================================
            with tc.tile_pool(name="p5sb", bufs=1) as sb5, tc.tile_pool(
                name="p5ps", bufs=1, space="PSUM"
            ) as ps5, nc.named_scope("p3_tail"):
                plog = ps5.tile([B_LOC, CLS], F32, tag="plog")
                nc.tensor.matmul(plog[:], yt_aug[:], w2_sb[:], start=True, stop=True)
                logit_sb = sb5.tile([B_LOC, CLS], F32)
                nc.scalar.copy(logit_sb[:], plog[:])
                nc.sync.dma_start(logits_d[:], logit_sb[:])
                # logits read from PSUM on DVE in parallel with the ACT copy
                nc.vector.tensor_copy(merged[:, 0:CLS], plog[:])

                # softmax over the 16 features (free dim).  No max-subtract:
                # |merged| <= ~6 here, exp() is safely in range, and softmax
                # is shift-invariant so the result matches the reference.
                esb = sb5.tile([B_LOC, 4 * CLS], F32)
                ssum = sb5.tile([B_LOC, 1], F32)
                nc.scalar.activation(
                    esb[:],
                    merged[:],
                    mybir.ActivationFunctionType.Exp,
                    accum_out=ssum[:],
                )
                rinv = sb5.tile([B_LOC, 1], F32)
                nc.vector.reciprocal(rinv[:], ssum[:])

                # softmax result with a ones column appended (becomes the
                # bias row after the transpose)
                smx = sb5.tile([B_LOC, 4 * CLS + 1], F32)
                nc.vector.tensor_scalar_mul(smx[:, 0 : 4 * CLS], esb[:], rinv[:])
                nc.vector.tensor_copy(
                    smx[:, 4 * CLS : 4 * CLS + 1], ones_col[0:B_LOC, 0:1]
                )

                # x_merge = smx @ cls_w.T + cls_b  (via transposed smx + aug)
                pmt = ps5.tile([4 * CLS + 1, B_LOC], F32, tag="pmt")
                nc.tensor.transpose(pmt[:], smx[:], identf[0:B_LOC, 0:B_LOC])
                mt_aug = sb5.tile([4 * CLS + 1, B_LOC], F32)
                nc.scalar.copy(mt_aug[:], pmt[:])

                pxm = ps5.tile([B_LOC, CLS], F32, tag="pxm")
                nc.tensor.matmul(pxm[:], mt_aug[:], wc_sb[:], start=True, stop=True)
                xm_sb = sb5.tile([B_LOC, CLS], F32)
                nc.scalar.copy(xm_sb[:], pxm[:])
                nc.sync.dma_start(xmerge_d[:], xm_sb[:])

    return nc


_NC_CACHE = None


def _get_nc():
    global _NC_CACHE
    if _NC_CACHE is None:
        _NC_CACHE = _build_nc()
    return _NC_CACHE


def _make_in_maps(inputs):
    np_mm = np.dtype(mybir.dt.np(MM_DT))

    x1 = np.ascontiguousarray(inputs["x1"], dtype=np.float32).reshape(B, C, L)
    x2 = np.ascontiguousarray(inputs["x2"], dtype=np.float32).reshape(B, C, L)
    x3 = np.ascontiguousarray(inputs["x3"], dtype=np.float32).reshape(B, C, L)
    # (B, L, M) concat + transpose, pad L to 3200, relayout to [B, 128, 25, M]
    xcat = np.concatenate([x1, x2, x3], axis=1).transpose(0, 2, 1)
    xpad = np.zeros((B, LC * 128, M), dtype=np_mm)
    xpad[:, :L] = xcat.astype(np_mm)
    xt_host = np.ascontiguousarray(
        xpad.reshape(B, LC, 128, M).transpose(0, 2, 1, 3)
    )

    x11 = np.ascontiguousarray(inputs["x11"], dtype=np.float32)
    x21 = np.ascontiguousarray(inputs["x21"], dtype=np.float32)
    x31 = np.ascontiguousarray(inputs["x31"], dtype=np.float32)
    fc0_w = np.asarray(inputs["fc0_w"], dtype=np.float32)
    fc0_b = np.asarray(inputs["fc0_b"], dtype=np.float32)
    fc1_w = np.asarray(inputs["fc1_w"], dtype=np.float32)
    fc1_b = np.asarray(inputs["fc1_b"], dtype=np.float32)
    fc2_w = np.asarray(inputs["fc2_w"], dtype=np.float32)
    fc2_b = np.asarray(inputs["fc2_b"], dtype=np.float32)
    cls_w = np.asarray(inputs["cls_w"], dtype=np.float32)
    cls_b = np.asarray(inputs["cls_b"], dtype=np.float32)

    # fuse fc0+fc1 (both linear; dropout is identity at inference), then
    # apply the symmetric fold (fp32 fold, cast to fp16 at the end)
    Wc = fc1_w @ fc0_w  # (64, 21609)
    bc1 = (fc1_w @ fc0_b + fc1_b).reshape(1, HID)
    Wcr = Wc.reshape(HID, M, M)
    WcA = Wcr[:, :MA, :].copy()
    WcA[:, :, MA:] += Wcr[:, MA:, :MA].transpose(0, 2, 1)
    wA_host = np.ascontiguousarray(WcA.transpose(1, 2, 0).astype(np_mm))  # [m, n, y]
    w4_host = np.ascontiguousarray(
        Wcr[:, MA:, MA:].transpose(1, 2, 0).astype(np_mm)
    )  # [a, b4, y]

    w2t = np.ascontiguousarray(
        np.concatenate([fc2_w.T, fc2_b.reshape(1, CLS)], axis=0)
    )
    wct = np.ascontiguousarray(
        np.concatenate([cls_w.T, cls_b.reshape(1, CLS)], axis=0)
    )

    in_maps = []
    for c in range(N_CORES):
        sl = slice(B_LOC * c, B_LOC * (c + 1))
        in_maps.append(
            {
                "xall": np.ascontiguousarray(xt_host[sl]),
                "x11": np.ascontiguousarray(x11[sl]),
                "x21": np.ascontiguousarray(x21[sl]),
                "x31": np.ascontiguousarray(x31[sl]),
                "wA": wA_host,
                "w4": w4_host,
                "bc1": np.ascontiguousarray(bc1),
                "w2t": w2t,
                "wct": wct,
            }
        )
    return in_maps


def run(inputs, trace=False, **kwargs):
    nc = _get_nc()
    in_maps = _make_in_maps(inputs)
    res = run_bass_kernel_spmd(nc, in_maps, CORE_IDS, trace=trace, **kwargs)
    logits = np.concatenate(
        [np.asarray(res.results[c]["logits"], dtype=np.float32) for c in CORE_IDS]
    )
    x_merge = np.concatenate(
        [np.asarray(res.results[c]["x_merge"], dtype=np.float32) for c in CORE_IDS]
    )
    return (logits, x_merge), res


def kernel(**inputs):
    (logits, x_merge), _ = run(inputs, trace=False)
    return logits, x_merge
